# revision 1
# baseline (speedup 1.0000x reference)
"""Trainium2 Bass kernel for retrieval-knn attention classifier (nn_MA_51866025067137).

Strategy (8 NeuronCores):
  Phase 1 — memory_keys sharded along N (12800 keys/core, padded 100000->102400
  with dummy rows).  Each core computes cosine-similarity ranking values for all
  256 queries against its shard (fp32r matmuls on the PE; keys pre-normalized so
  the matmul directly yields cosine ranking values) and extracts its local
  top-32 (value, index) per query with DVE max8/max_index/match_replace, using a
  low-9-mantissa-bit packing trick to recover in-segment indices.
  Host — merges the 8x40 candidates per row, re-scores them exactly in fp32,
  and gathers the global top-32 key vectors.
  Phase 2 — batch sharded (32 queries/core): memory-attention module
  (tanh(qWq + knnWm + b) -> scores -> softmax -> weighted sum) and classifier,
  via small fp32r matmuls; the softmax-weighted sum is a block-diagonal matmul.
"""

import numpy as np

import concourse.bacc as bacc
import concourse.mybir as mybir
from concourse.tile import TileContext
from concourse.bass_utils import run_bass_kernel_spmd
from concourse.masks import make_identity

# problem dims (hardcoded per harness contract)
B, N, D = 256, 100000, 512
A, C, K = 256, 100, 32
NC_CORES = 8
NPAD = 102400             # 8 * 12800
SHARD = NPAD // NC_CORES  # 12800
CHUNK = 512               # keys per inner loop step
NCHUNK = SHARD // CHUNK   # 25
SEG = 512                 # max8 segment width (9-bit in-segment index)
NSEG = SHARD // SEG       # 25
L1W = NSEG * 8            # 200
BROWS = B // NC_CORES     # 32 rows per core in phase 2
KLOC = 40                 # local candidates per core per row
CAND = NC_CORES * KLOC    # 320 merged candidates per row

f32 = mybir.dt.float32
f32r = mybir.dt.float32r
u32 = mybir.dt.uint32

_PH1 = None
_PH2 = None


def _build_phase1():
    nc = bacc.Bacc("TRN2", target_bir_lowering=False)
    khatT = nc.dram_tensor("khatT", [NCHUNK, 128, 4 * CHUNK], f32r, kind="ExternalInput")
    qT = nc.dram_tensor("qT", [D, B], f32r, kind="ExternalInput")
    win_out = nc.dram_tensor("win", [B, KLOC], f32, kind="ExternalOutput")
    pos_out = nc.dram_tensor("pos", [B, KLOC], u32, kind="ExternalOutput")

    with TileContext(nc) as tc:
        with (
            tc.tile_pool(name="const", bufs=1) as constp,
            tc.tile_pool(name="qpool", bufs=1) as qpool,
            tc.tile_pool(name="keys", bufs=6) as keyp,
            tc.tile_pool(name="packed", bufs=8) as packp,
            tc.tile_pool(name="l1", bufs=1) as l1p,
            tc.tile_pool(name="small", bufs=1) as smallp,
            tc.tile_pool(name="psum", bufs=2, space="PSUM") as psump,
        ):
            # constants: AND-mask (0xFFFFFE00) per partition; iota 0..511
            mask_t = constp.tile([128, 1], u32, tag="mask")
            nc.vector.memset(mask_t[:], 0xFFFFFE00)
            iota_t = constp.tile([128, CHUNK], u32, tag="iota")
            nc.gpsimd.iota(iota_t[:], pattern=[[1, CHUNK]], base=0,
                           channel_multiplier=0)

            # load qT and relu in place
            qTr = []
            for dc in range(4):
                t = qpool.tile([128, B], f32r, tag=f"qt{dc}")
                nc.sync.dma_start(out=t[:], in_=qT[dc * 128:(dc + 1) * 128, :])
                nc.scalar.activation(t[:], t[:], mybir.ActivationFunctionType.Relu)
                qTr.append(t)

            L1 = [l1p.tile([128, L1W], f32, tag=f"l1_{qt}", name=f"l1_{qt}") for qt in range(2)]

            for c in range(NCHUNK):
                kt = keyp.tile([128, 4 * CHUNK], f32r, tag="kt")
                nc.sync.dma_start(out=kt[:], in_=khatT[c, :, :])
                for qt in range(2):
                    ps = psump.tile([128, CHUNK], f32, tag=f"sim{qt}")
                    for dc in range(4):
                        nc.tensor.matmul(
                            ps[:],
                            lhsT=qTr[dc][:, qt * 128:(qt + 1) * 128],
                            rhs=kt[:, dc * CHUNK:(dc + 1) * CHUNK],
                            start=(dc == 0), stop=(dc == 3),
                        )
                    # evict (ACT), pack on GPSIMD: packed = (sim & mask) | iota
                    ev = packp.tile([128, CHUNK], f32, tag=f"ev{qt}")
                    nc.scalar.copy(out=ev[:], in_=ps[:])
                    pk = packp.tile([128, CHUNK], f32, tag=f"pk{qt}")
                    nc.vector.scalar_tensor_tensor(
                        out=pk[:].bitcast(u32), in0=ev[:].bitcast(u32),
                        scalar=mask_t[:], in1=iota_t[:],
                        op0=mybir.AluOpType.bitwise_and,
                        op1=mybir.AluOpType.bitwise_or,
                    )
                    nc.vector.max(out=L1[qt][:, c * 8:(c + 1) * 8], in_=pk[:])

            # extraction: 5 rounds of top-8 from L1 (400 wide)
            for qt in range(2):
                win = smallp.tile([128, KLOC], f32, tag=f"win{qt}")
                pos = smallp.tile([128, KLOC], u32, tag=f"pos{qt}")
                for r in range(5):
                    w8 = win[:, r * 8:(r + 1) * 8]
                    nc.vector.max(out=w8, in_=L1[qt][:])
                    nc.vector.max_index(out=pos[:, r * 8:(r + 1) * 8],
                                        in_max=w8, in_values=L1[qt][:])
                    if r < 4:
                        nc.vector.match_replace(out=L1[qt][:], in_to_replace=w8,
                                                in_values=L1[qt][:],
                                                imm_value=-3.0e38)
                nc.sync.dma_start(out=win_out[qt * 128:(qt + 1) * 128, :], in_=win[:])
                nc.sync.dma_start(out=pos_out[qt * 128:(qt + 1) * 128, :], in_=pos[:])
    nc.finalize()
    return nc


def _build_phase2():
    nc = bacc.Bacc("TRN2", target_bir_lowering=False)
    qT_in = nc.dram_tensor("qT", [D, BROWS], f32r, kind="ExternalInput")       # pre-relu
    knn_in = nc.dram_tensor("knn", [BROWS * K, D], f32r, kind="ExternalInput")
    knnT_in = nc.dram_tensor("knnT", [D, BROWS * K], f32r, kind="ExternalInput")
    Wq_in = nc.dram_tensor("Wq", [D, A], f32r, kind="ExternalInput")
    Wm_in = nc.dram_tensor("Wm", [D, A], f32r, kind="ExternalInput")
    Ws_in = nc.dram_tensor("Ws", [A, 1], f32r, kind="ExternalInput")
    bqm_in = nc.dram_tensor("bqm", [A, 1], f32, kind="ExternalInput")          # bq+bm
    Wc_in = nc.dram_tensor("Wc", [2 * D, C], f32r, kind="ExternalInput")
    out_d = nc.dram_tensor("out", [BROWS, C], f32, kind="ExternalOutput")      # +bc host
    escratch = nc.dram_tensor("escratch", [1, BROWS * K], f32)                 # bounce

    NCD = BROWS * K  # 1024

    with TileContext(nc) as tc:
        with (
            tc.tile_pool(name="big", bufs=1) as bigp,
            tc.tile_pool(name="small", bufs=1) as smallp,
            tc.tile_pool(name="psum", bufs=1, space="PSUM") as psump,
        ):
            # ---- load inputs (M-padded tiles to satisfy fp32r col_grp=0xf) ----
            qT = [smallp.tile([128, 128], f32r, tag=f"qT{dc}", name=f"qTt{dc}") for dc in range(4)]
            for dc in range(4):
                nc.vector.memset(qT[dc][:].bitcast(u32), 0)
                nc.sync.dma_start(out=qT[dc][:, :BROWS],
                                  in_=qT_in[dc * 128:(dc + 1) * 128, :])
                nc.scalar.activation(qT[dc][:, :BROWS], qT[dc][:, :BROWS],
                                     mybir.ActivationFunctionType.Relu)
            knnall = bigp.tile([128, 8 * D], f32r, tag="knnall")
            nc.sync.dma_start(out=knnall[:].rearrange("p (t d) -> p t d", t=8),
                              in_=knn_in[:].rearrange("(t p) d -> p t d", p=128))
            knn = [knnall[:, t * D:(t + 1) * D] for t in range(8)]
            knnTall = bigp.tile([128, 4 * NCD], f32r, tag="knnTall")
            nc.sync.dma_start(out=knnTall[:].rearrange("p (dc c) -> p dc c", dc=4),
                              in_=knnT_in[:].rearrange("(dc p) c -> p dc c", p=128))
            knnT = [knnTall[:, dc * NCD:(dc + 1) * NCD] for dc in range(4)]
            Wqall = smallp.tile([128, 4 * A], f32r, tag="Wqall")
            nc.sync.dma_start(out=Wqall[:].rearrange("p (dc a) -> p dc a", dc=4),
                              in_=Wq_in[:].rearrange("(dc p) a -> p dc a", p=128))
            Wq = [Wqall[:, dc * A:(dc + 1) * A] for dc in range(4)]
            Wmall = smallp.tile([128, 4 * A], f32r, tag="Wmall")
            nc.sync.dma_start(out=Wmall[:].rearrange("p (dc a) -> p dc a", dc=4),
                              in_=Wm_in[:].rearrange("(dc p) a -> p dc a", p=128))
            Wm = [Wmall[:, dc * A:(dc + 1) * A] for dc in range(4)]
            Ws = [smallp.tile([128, 128], f32r, tag=f"Ws{at}", name=f"Wst{at}") for at in range(2)]
            bqm = [smallp.tile([128, 1], f32, tag=f"bqm{at}", name=f"bqmt{at}") for at in range(2)]
            for at in range(2):
                nc.vector.memset(Ws[at][:].bitcast(u32), 0)
                nc.sync.dma_start(out=Ws[at][:, :1],
                                  in_=Ws_in[at * 128:(at + 1) * 128, :])
                nc.sync.dma_start(out=bqm[at][:],
                                  in_=bqm_in[at * 128:(at + 1) * 128, :])
            Wcall = smallp.tile([128, 8 * C], f32r, tag="Wcall")
            nc.sync.dma_start(out=Wcall[:].rearrange("p (m j) -> p m j", m=8),
                              in_=Wc_in[:].rearrange("(m p) j -> p m j", p=128))
            Wc = [Wcall[:, m * C:(m + 1) * C] for m in range(8)]
            ones = smallp.tile([128, 2], f32r, tag="ones")
            nc.vector.memset(ones[:].bitcast(u32), 0)
            nc.vector.memset(ones[:, :1].bitcast(u32), 0x3F800000)
            # mask4[p, j] = 1.0 iff j == p // 32
            mask4 = smallp.tile([128, 4], f32, tag="mask4")
            nc.vector.memset(mask4[:], 1.0)
            nc.gpsimd.affine_select(out=mask4[:], in_=mask4[:],
                                    compare_op=mybir.AluOpType.is_ge, fill=0.0,
                                    base=0, pattern=[[-32, 4]], channel_multiplier=1)
            nc.gpsimd.affine_select(out=mask4[:], in_=mask4[:],
                                    compare_op=mybir.AluOpType.is_ge, fill=0.0,
                                    base=31, pattern=[[32, 4]], channel_multiplier=-1)
            ident = smallp.tile([128, 128], f32, tag="ident")
            make_identity(nc, ident[:])

            # ---- qprojT [2][128a, 32] ----
            qprojT = [smallp.tile([128, BROWS], f32, tag=f"qp{at}", name=f"qpt{at}") for at in range(2)]
            for at in range(2):
                ps = psump.tile([128, BROWS], f32, tag="ps_a")
                for dc in range(4):
                    nc.tensor.matmul(
                        ps[:],
                        lhsT=Wq[dc][:, at * 128:(at + 1) * 128],
                        rhs=qT[dc][:, :BROWS],
                        start=(dc == 0), stop=(dc == 3))
                nc.scalar.copy(out=qprojT[at][:], in_=ps[:])

            # ---- hT = tanh(kprojT + qprojT_bcast + bqm) ; scores ----
            sc_ps = psump.tile([128, NCD], f32, tag="ps_sc")
            for at in range(2):
                kp = psump.tile([128, NCD], f32, tag="ps_kp", bufs=2)
                for dc in range(4):
                    for half in range(2):
                        nc.tensor.matmul(
                            kp[:, half * 512:(half + 1) * 512],
                            lhsT=Wm[dc][:, at * 128:(at + 1) * 128],
                            rhs=knnT[dc][:, half * 512:(half + 1) * 512],
                            start=(dc == 0), stop=(dc == 3))
                hT = bigp.tile([128, NCD], f32r, tag=f"hT{at}")
                qb = qprojT[at][:, :, None].to_broadcast([128, BROWS, K])
                nc.vector.tensor_tensor(
                    hT[:].rearrange("p (q k) -> p q k", k=K),
                    kp[:].rearrange("p (q k) -> p q k", k=K),
                    qb, mybir.AluOpType.add)
                nc.scalar.activation(hT[:], hT[:], mybir.ActivationFunctionType.Tanh,
                                     bias=bqm[at][:])
                for half in range(2):
                    nc.tensor.matmul(
                        sc_ps[:, half * 512:(half + 1) * 512],
                        lhsT=Ws[at][:],
                        rhs=hT[:, half * 512:(half + 1) * 512],
                        start=(at == 0), stop=(at == 1))
            e_row = smallp.tile([1, NCD], f32, tag="e_row")
            nc.scalar.activation(e_row[:], sc_ps[:1, :],
                                 mybir.ActivationFunctionType.Exp)
            # bounce through DRAM to redistribute [1, 1024] -> [128, 8]
            nc.sync.dma_start(out=escratch[:, :], in_=e_row[:, :])
            e_col = smallp.tile([128, 8], f32, tag="e_col")
            nc.sync.dma_start(out=e_col[:],
                              in_=escratch[0, :].rearrange("(t p) -> p t", p=128))

            # ---- block-diag softmax weights (M-padded), den, attended ----
            w2 = [bigp.tile([128, 128], f32r, tag=f"w2_{t}", name=f"w2t{t}") for t in range(8)]
            for t in range(8):
                nc.vector.memset(w2[t][:].bitcast(u32), 0)
                nc.vector.tensor_scalar_mul(w2[t][:, 4 * t:4 * t + 4], mask4[:],
                                            e_col[:, t:t + 1])
            den_ps = psump.tile([128, 2], f32, tag="ps_a")
            for t in range(8):
                nc.tensor.matmul(den_ps[:], lhsT=w2[t][:], rhs=ones[:],
                                 start=(t == 0), stop=(t == 7))
            att_ps = psump.tile([128, D], f32, tag="ps_kp", bufs=2)
            for t in range(8):
                nc.tensor.matmul(att_ps[:], lhsT=w2[t][:], rhs=knn[t],
                                 start=(t == 0), stop=(t == 7))
            rden = smallp.tile([BROWS, 1], f32, tag="rden")
            nc.vector.reciprocal(rden[:], den_ps[:BROWS, :1])
            att = smallp.tile([BROWS, D], f32, tag="att_sb")
            nc.vector.tensor_scalar_mul(att[:], att_ps[:BROWS, :], rden[:])

            # ---- attendedT via PE transpose (plain fp32) ----
            attT = [smallp.tile([128, 128], f32r, tag=f"attT{dc}", name=f"attTt{dc}") for dc in range(4)]
            for dc in range(4):
                tp = psump.tile([128, BROWS], f32, tag="ps_a")
                nc.tensor.transpose(tp[:], att[:, dc * 128:(dc + 1) * 128],
                                    ident[:BROWS, :BROWS])
                nc.vector.memset(attT[dc][:].bitcast(u32), 0)
                nc.scalar.copy(out=attT[dc][:, :BROWS], in_=tp[:])

            # ---- classifier ----
            out_ps = psump.tile([128, C], f32, tag="ps_out")
            for m in range(8):
                lhsT = qT[m] if m < 4 else attT[m - 4]
                nc.tensor.matmul(out_ps[:], lhsT=lhsT[:], rhs=Wc[m],
                                 start=(m == 0), stop=(m == 7))
            out_sb = smallp.tile([BROWS, C], f32, tag="out_sb")
            nc.scalar.copy(out=out_sb[:], in_=out_ps[:BROWS, :])
            nc.sync.dma_start(out=out_d[:, :], in_=out_sb[:])
    nc.finalize()
    return nc


def _phase1_nc():
    global _PH1
    if _PH1 is None:
        _PH1 = _build_phase1()
    return _PH1


def _phase2_nc():
    global _PH2
    if _PH2 is None:
        _PH2 = _build_phase2()
    return _PH2


def kernel(query_feat, memory_keys, Wq, bq, Wm, bm, Ws, bs, Wc, bc):
    query_feat = np.asarray(query_feat, np.float32)
    memory_keys = np.asarray(memory_keys, np.float32)

    # ---- host prep: pad + normalize + transpose + shard keys ----
    kn = np.sqrt((memory_keys ** 2).sum(axis=1))
    khat = memory_keys * (1.0 / kn)[:, None]
    pad = np.full((NPAD - N, D), -1.0 / np.sqrt(D), np.float32)
    khat_pad = np.concatenate([khat.astype(np.float32), pad], axis=0)
    qT_full = np.ascontiguousarray(query_feat.T)  # [512, 256]

    ph1 = _phase1_nc()
    in_maps = []
    for c in range(NC_CORES):
        sh = khat_pad[c * SHARD:(c + 1) * SHARD]          # [12800, 512]
        arr = np.ascontiguousarray(
            sh.reshape(NCHUNK, CHUNK, 4, 128).transpose(0, 3, 2, 1)
        ).reshape(NCHUNK, 128, 4 * CHUNK)
        in_maps.append({"khatT": arr, "qT": qT_full})
    res1 = run_bass_kernel_spmd(ph1, in_maps, core_ids=list(range(NC_CORES)))

    # ---- host merge: recover indices, exact re-score of candidates ----
    all_gidx = np.zeros((B, NC_CORES, KLOC), np.int64)
    for c in range(NC_CORES):
        win = res1.results[c]["win"].view(np.uint32)
        pos = res1.results[c]["pos"].astype(np.int64)   # 0..399 in L1
        seg = pos // 8
        within = (win & np.uint32(0x1FF)).astype(np.int64)
        all_gidx[:, c, :] = seg * SEG + within + c * SHARD
    gidx = all_gidx.reshape(B, CAND)
    safe = np.minimum(gidx, N - 1)
    q32 = np.maximum(query_feat, 0)
    cand_keys = memory_keys[safe]                       # [256, 320, 512]
    dots = np.einsum("bd,bcd->bc", q32, cand_keys, optimize=True)
    cos = dots / np.maximum(
        np.linalg.norm(q32, axis=1)[:, None] * kn[safe], np.float32(1e-8))
    cos[gidx >= N] = -np.inf                            # mask dummy-pad hits
    order = np.argsort(-cos, axis=1, kind="stable")[:, :K]
    top_idx = np.take_along_axis(safe, order, axis=1)   # [256, 32]
    knn = memory_keys[top_idx]                          # [256, 32, 512]

    # ---- phase 2 (batch sharded) ----
    ph2 = _phase2_nc()
    bqm = (np.asarray(bq, np.float32) + np.asarray(bm, np.float32)).reshape(A, 1)
    Wq_a = np.ascontiguousarray(np.asarray(Wq, np.float32))
    Wm_a = np.ascontiguousarray(np.asarray(Wm, np.float32))
    Ws_a = np.ascontiguousarray(np.asarray(Ws, np.float32))
    Wc_a = np.ascontiguousarray(np.asarray(Wc, np.float32))
    in_maps2 = []
    for c in range(NC_CORES):
        rows = slice(c * BROWS, (c + 1) * BROWS)
        knn_c = knn[rows].reshape(BROWS * K, D)
        in_maps2.append({
            "qT": np.ascontiguousarray(query_feat[rows].T),
            "knn": np.ascontiguousarray(knn_c),
            "knnT": np.ascontiguousarray(knn_c.T),
            "Wq": Wq_a, "Wm": Wm_a, "Ws": Ws_a, "bqm": bqm, "Wc": Wc_a,
        })
    res2 = run_bass_kernel_spmd(ph2, in_maps2, core_ids=list(range(NC_CORES)))
    out = np.concatenate([res2.results[c]["out"] for c in range(NC_CORES)], axis=0)
    return (out + np.asarray(bc, np.float32)[None, :]).astype(np.float32)



# revision 7
# speedup vs baseline: 1.5973x; 1.5973x over previous
"""Trainium2 Bass kernel for retrieval-knn attention classifier (nn_MA_51866025067137).

Strategy (8 NeuronCores):
  Phase 1 — memory_keys sharded along N (12800 keys/core, padded 100000->102400).
  Keys/queries are quantized to small integers and fed to fp8e4 DoubleRow
  matmuls (2 rows/cycle on the PE).  A 3-row fp32r "bias" matmul adds
  BIG + iota*2^-9 so every similarity lands in one fp32 binade [2^14, 2^15)
  with its low 10 mantissa bits equal to the column index (sims are exact
  even integers, so the pack costs nothing).  Per-1024 window top-8 is then a
  single DVE max8 straight out of PSUM; the first 6400 columns instead go
  through ACT eviction + two GPSIMD topk calls (top-256 per 16-partition
  token with indices).  Candidate extraction/merging/re-scoring is done on
  the host, which gathers the exact global top-32 key vectors.
  Phase 2 — batch sharded (32 queries/core): memory-attention module
  (tanh(qWq + knnWm + b) -> scores -> softmax -> weighted sum) and classifier,
  via small fp32r matmuls; the softmax-weighted sum is a block-diagonal matmul.
"""

import numpy as np
import ml_dtypes

import concourse.bacc as bacc
import concourse.mybir as mybir
from concourse.tile import TileContext, add_dep_helper
from concourse.bass_utils import run_bass_kernel_spmd
from concourse.masks import make_identity

# problem dims (hardcoded per harness contract)
B, N, D = 256, 100000, 512
A, C, K = 256, 100, 32
NC_CORES = 8
NPAD = 102400             # 8 * 12800
SHARD = NPAD // NC_CORES  # 12800
WIN = 1024                # DVE max8 window
NWIN = 13                 # windows 0..11 full, 12 is half (512)
TOPW = 3200               # gpsimd topk region width (vocab = 16*3200 = 51200)
NREG = 2                  # topk regions per qt: cols [0, 6400)
TOPC = NREG * TOPW        # 6400 cols to ACT-evict per qt
BROWS = B // NC_CORES     # 32 rows per core in phase 2
BIG = 24576.0             # binade [2^14, 2^15); ulp 2^-9
ULP = 2.0 ** -9

f32 = mybir.dt.float32
f32r = mybir.dt.float32r
f8 = mybir.dt.float8e4
u32 = mybir.dt.uint32

_PH1 = None
_PH2 = None


def _u(i):
    return i.ins if hasattr(i, "ins") else i


def _build_phase1():
    nc = bacc.Bacc("TRN2", target_bir_lowering=False)
    kT_d = nc.dram_tensor("kT", [25, 128, 2, 2, 512], f8, kind="ExternalInput")
    qT_d = nc.dram_tensor("qT", [128, 2, 2, 256], f8, kind="ExternalInput")
    bias_d = nc.dram_tensor("bias", [3, WIN], f32r, kind="ExternalInput")
    l1_d = nc.dram_tensor("l1", [2, 128, 56], f32, kind="ExternalOutput")
    tk_d = nc.dram_tensor("tk", [2, NREG, 128, 32], u32, kind="ExternalOutput")

    with TileContext(nc) as tc:
        with (
            tc.tile_pool(name="const", bufs=1) as constp,
            tc.tile_pool(name="keys", bufs=6) as keyp,
            tc.tile_pool(name="l1", bufs=1) as l1p,
            tc.tile_pool(name="psum", bufs=2, space="PSUM") as psump,
        ):
            qT = constp.tile([128, 2, 2, 256], f8, tag="qT", name="qT_t")
            nc.sync.dma_start(out=qT[:], in_=qT_d[:, :, :, :])
            ones3 = constp.tile([3, 128], f32r, tag="ones3", name="ones3_t")
            nc.vector.memset(ones3[:].bitcast(u32), 0x3F800000)
            bias = constp.tile([3, WIN], f32r, tag="bias", name="bias_t")
            nc.sync.dma_start(out=bias[:], in_=bias_d[:, :])

            # raw SBUF for gpsimd topk (per qt) + its output
            sims_sb = [[nc.alloc_sbuf_tensor(f"sims_sb{qt}_{r}", [128, TOPW], f32)
                        for r in range(NREG)] for qt in range(2)]
            tk_sb = [[nc.alloc_sbuf_tensor(f"tk_sb{qt}_{r}", [128, 32], u32)
                      for r in range(NREG)] for qt in range(2)]

            L1 = [l1p.tile([128, 56], f32, tag=f"l1_{qt}", name=f"l1_{qt}")
                  for qt in range(2)]
            evicts = [[], []]   # per qt: ACT evict instructions feeding topk

            for w in range(NWIN):
                wcols = 512 if w == 12 else WIN
                nchunk = wcols // 512
                kt = []
                for h in range(nchunk):
                    t = keyp.tile([128, 2, 2, 512], f8, tag="kt", name="kt_t")
                    nc.sync.dma_start(out=t[:], in_=kT_d[2 * w + h, :, :, :, :])
                    kt.append(t)
                for qt in range(2):
                    ps = psump.tile([128, WIN], f32, tag=f"win{qt}", name=f"ps{qt}")
                    for h in range(nchunk):
                        sl = slice(h * 512, (h + 1) * 512)
                        for dc in range(2):
                            nc.tensor.matmul(
                                ps[:, sl],
                                lhsT=qT[:, dc, :, qt * 128:(qt + 1) * 128],
                                rhs=kt[h][:, dc, :, :],
                                start=(dc == 0), stop=False,
                                perf_mode=mybir.MatmulPerfMode.DoubleRow)
                        nc.tensor.matmul(
                            ps[:, sl], lhsT=ones3[:],
                            rhs=bias[:, h * 512:(h + 1) * 512],
                            start=False, stop=True)
                    if w < 6:
                        # full window -> topk staging buffers (regions of 3200)
                        base = w * WIN
                        for r in range(NREG):
                            lo = max(base, r * TOPW)
                            hi = min(base + WIN, (r + 1) * TOPW)
                            if lo < hi:
                                ev = nc.scalar.copy(
                                    out=sims_sb[qt][r][:, lo - r * TOPW:hi - r * TOPW],
                                    in_=ps[:, lo - base:hi - base])
                                evicts[qt].append(ev)
                    elif w == 6:
                        ev = nc.scalar.copy(out=sims_sb[qt][1][:, 6 * WIN - TOPW:TOPW],
                                            in_=ps[:, :256])
                        evicts[qt].append(ev)
                        nc.vector.max(out=L1[qt][:, 0:8], in_=ps[:, 256:])
                    elif w < 12:
                        nc.vector.max(out=L1[qt][:, 8 * (w - 6):8 * (w - 5)], in_=ps[:])
                    else:
                        nc.vector.max(out=L1[qt][:, 48:56], in_=ps[:, :512])

            for qt in range(2):
                for r in range(NREG):
                    tki = nc.gpsimd.topk(
                        tk_sb[qt][r][:], sims_sb[qt][r][:],
                        tokens=8, vocab_size=16 * TOPW, k=256)
                    for ev in evicts[qt]:
                        add_dep_helper(_u(tki), _u(ev), reason="topk waits evicts")
                    do = nc.sync.dma_start(out=tk_d[qt, r, :, :],
                                           in_=tk_sb[qt][r][:])
                    add_dep_helper(_u(do), _u(tki), reason="tk out waits topk")
                nc.sync.dma_start(out=l1_d[qt, :, :], in_=L1[qt][:])
    nc.finalize()
    return nc


def _build_phase2():
    nc = bacc.Bacc("TRN2", target_bir_lowering=False)
    qT_in = nc.dram_tensor("qT", [D, BROWS], f32r, kind="ExternalInput")       # pre-relu
    knn_in = nc.dram_tensor("knn", [BROWS * K, D], f32r, kind="ExternalInput")
    knnT_in = nc.dram_tensor("knnT", [D, BROWS * K], f32r, kind="ExternalInput")
    Wq_in = nc.dram_tensor("Wq", [D, A], f32r, kind="ExternalInput")
    Wm_in = nc.dram_tensor("Wm", [D, A], f32r, kind="ExternalInput")
    Ws_in = nc.dram_tensor("Ws", [A, 1], f32r, kind="ExternalInput")
    bqm_in = nc.dram_tensor("bqm", [A, 1], f32, kind="ExternalInput")          # bq+bm
    Wc_in = nc.dram_tensor("Wc", [2 * D, C], f32r, kind="ExternalInput")
    out_d = nc.dram_tensor("out", [BROWS, C], f32, kind="ExternalOutput")      # +bc host
    escratch = nc.dram_tensor("escratch", [1, BROWS * K], f32)                 # bounce

    NCD = BROWS * K  # 1024

    with TileContext(nc) as tc:
        with (
            tc.tile_pool(name="big", bufs=1) as bigp,
            tc.tile_pool(name="small", bufs=1) as smallp,
            tc.tile_pool(name="psum", bufs=1, space="PSUM") as psump,
        ):
            # ---- load inputs (M-padded tiles to satisfy fp32r col_grp=0xf) ----
            qT = [smallp.tile([128, 128], f32r, tag=f"qT{dc}", name=f"qTt{dc}") for dc in range(4)]
            for dc in range(4):
                nc.vector.memset(qT[dc][:].bitcast(u32), 0)
                nc.sync.dma_start(out=qT[dc][:, :BROWS],
                                  in_=qT_in[dc * 128:(dc + 1) * 128, :])
                nc.scalar.activation(qT[dc][:, :BROWS], qT[dc][:, :BROWS],
                                     mybir.ActivationFunctionType.Relu)
            knnall = bigp.tile([128, 8 * D], f32r, tag="knnall")
            nc.sync.dma_start(out=knnall[:].rearrange("p (t d) -> p t d", t=8),
                              in_=knn_in[:].rearrange("(t p) d -> p t d", p=128))
            knn = [knnall[:, t * D:(t + 1) * D] for t in range(8)]
            knnTall = bigp.tile([128, 4 * NCD], f32r, tag="knnTall")
            nc.sync.dma_start(out=knnTall[:].rearrange("p (dc c) -> p dc c", dc=4),
                              in_=knnT_in[:].rearrange("(dc p) c -> p dc c", p=128))
            knnT = [knnTall[:, dc * NCD:(dc + 1) * NCD] for dc in range(4)]
            Wqall = smallp.tile([128, 4 * A], f32r, tag="Wqall")
            nc.sync.dma_start(out=Wqall[:].rearrange("p (dc a) -> p dc a", dc=4),
                              in_=Wq_in[:].rearrange("(dc p) a -> p dc a", p=128))
            Wq = [Wqall[:, dc * A:(dc + 1) * A] for dc in range(4)]
            Wmall = smallp.tile([128, 4 * A], f32r, tag="Wmall")
            nc.sync.dma_start(out=Wmall[:].rearrange("p (dc a) -> p dc a", dc=4),
                              in_=Wm_in[:].rearrange("(dc p) a -> p dc a", p=128))
            Wm = [Wmall[:, dc * A:(dc + 1) * A] for dc in range(4)]
            Ws = [smallp.tile([128, 128], f32r, tag=f"Ws{at}", name=f"Wst{at}") for at in range(2)]
            bqm = [smallp.tile([128, 1], f32, tag=f"bqm{at}", name=f"bqmt{at}") for at in range(2)]
            for at in range(2):
                nc.vector.memset(Ws[at][:].bitcast(u32), 0)
                nc.sync.dma_start(out=Ws[at][:, :1],
                                  in_=Ws_in[at * 128:(at + 1) * 128, :])
                nc.sync.dma_start(out=bqm[at][:],
                                  in_=bqm_in[at * 128:(at + 1) * 128, :])
            Wcall = smallp.tile([128, 8 * C], f32r, tag="Wcall")
            nc.sync.dma_start(out=Wcall[:].rearrange("p (m j) -> p m j", m=8),
                              in_=Wc_in[:].rearrange("(m p) j -> p m j", p=128))
            Wc = [Wcall[:, m * C:(m + 1) * C] for m in range(8)]
            ones = smallp.tile([128, 2], f32r, tag="ones")
            nc.vector.memset(ones[:].bitcast(u32), 0)
            nc.vector.memset(ones[:, :1].bitcast(u32), 0x3F800000)
            # mask4[p, j] = 1.0 iff j == p // 32
            mask4 = smallp.tile([128, 4], f32, tag="mask4")
            nc.vector.memset(mask4[:], 1.0)
            nc.gpsimd.affine_select(out=mask4[:], in_=mask4[:],
                                    compare_op=mybir.AluOpType.is_ge, fill=0.0,
                                    base=0, pattern=[[-32, 4]], channel_multiplier=1)
            nc.gpsimd.affine_select(out=mask4[:], in_=mask4[:],
                                    compare_op=mybir.AluOpType.is_ge, fill=0.0,
                                    base=31, pattern=[[32, 4]], channel_multiplier=-1)
            ident = smallp.tile([128, 128], f32, tag="ident")
            make_identity(nc, ident[:])

            # ---- qprojT [2][128a, 32] ----
            qprojT = [smallp.tile([128, BROWS], f32, tag=f"qp{at}", name=f"qpt{at}") for at in range(2)]
            for at in range(2):
                ps = psump.tile([128, BROWS], f32, tag="ps_a")
                for dc in range(4):
                    nc.tensor.matmul(
                        ps[:],
                        lhsT=Wq[dc][:, at * 128:(at + 1) * 128],
                        rhs=qT[dc][:, :BROWS],
                        start=(dc == 0), stop=(dc == 3))
                nc.scalar.copy(out=qprojT[at][:], in_=ps[:])

            # ---- hT = tanh(kprojT + qprojT_bcast + bqm) ; scores ----
            sc_ps = psump.tile([128, NCD], f32, tag="ps_sc")
            for at in range(2):
                kp = psump.tile([128, NCD], f32, tag="ps_kp", bufs=2)
                for dc in range(4):
                    for half in range(2):
                        nc.tensor.matmul(
                            kp[:, half * 512:(half + 1) * 512],
                            lhsT=Wm[dc][:, at * 128:(at + 1) * 128],
                            rhs=knnT[dc][:, half * 512:(half + 1) * 512],
                            start=(dc == 0), stop=(dc == 3))
                hT = bigp.tile([128, NCD], f32r, tag=f"hT{at}")
                qb = qprojT[at][:, :, None].to_broadcast([128, BROWS, K])
                nc.vector.tensor_tensor(
                    hT[:].rearrange("p (q k) -> p q k", k=K),
                    kp[:].rearrange("p (q k) -> p q k", k=K),
                    qb, mybir.AluOpType.add)
                nc.scalar.activation(hT[:], hT[:], mybir.ActivationFunctionType.Tanh,
                                     bias=bqm[at][:])
                for half in range(2):
                    nc.tensor.matmul(
                        sc_ps[:, half * 512:(half + 1) * 512],
                        lhsT=Ws[at][:],
                        rhs=hT[:, half * 512:(half + 1) * 512],
                        start=(at == 0), stop=(at == 1))
            e_row = smallp.tile([1, NCD], f32, tag="e_row")
            nc.scalar.activation(e_row[:], sc_ps[:1, :],
                                 mybir.ActivationFunctionType.Exp)
            # bounce through DRAM to redistribute [1, 1024] -> [128, 8]
            nc.sync.dma_start(out=escratch[:, :], in_=e_row[:, :])
            e_col = smallp.tile([128, 8], f32, tag="e_col")
            nc.sync.dma_start(out=e_col[:],
                              in_=escratch[0, :].rearrange("(t p) -> p t", p=128))

            # ---- block-diag softmax weights (M-padded), den, attended ----
            w2 = [bigp.tile([128, 128], f32r, tag=f"w2_{t}", name=f"w2t{t}") for t in range(8)]
            for t in range(8):
                nc.vector.memset(w2[t][:].bitcast(u32), 0)
                nc.vector.tensor_scalar_mul(w2[t][:, 4 * t:4 * t + 4], mask4[:],
                                            e_col[:, t:t + 1])
            den_ps = psump.tile([128, 2], f32, tag="ps_a")
            for t in range(8):
                nc.tensor.matmul(den_ps[:], lhsT=w2[t][:], rhs=ones[:],
                                 start=(t == 0), stop=(t == 7))
            att_ps = psump.tile([128, D], f32, tag="ps_kp", bufs=2)
            for t in range(8):
                nc.tensor.matmul(att_ps[:], lhsT=w2[t][:], rhs=knn[t],
                                 start=(t == 0), stop=(t == 7))
            rden = smallp.tile([BROWS, 1], f32, tag="rden")
            nc.vector.reciprocal(rden[:], den_ps[:BROWS, :1])
            att = smallp.tile([BROWS, D], f32, tag="att_sb")
            nc.vector.tensor_scalar_mul(att[:], att_ps[:BROWS, :], rden[:])

            # ---- attendedT via PE transpose (plain fp32) ----
            attT = [smallp.tile([128, 128], f32r, tag=f"attT{dc}", name=f"attTt{dc}") for dc in range(4)]
            for dc in range(4):
                tp = psump.tile([128, BROWS], f32, tag="ps_a")
                nc.tensor.transpose(tp[:], att[:, dc * 128:(dc + 1) * 128],
                                    ident[:BROWS, :BROWS])
                nc.vector.memset(attT[dc][:].bitcast(u32), 0)
                nc.scalar.copy(out=attT[dc][:, :BROWS], in_=tp[:])

            # ---- classifier ----
            out_ps = psump.tile([128, C], f32, tag="ps_out")
            for m in range(8):
                lhsT = qT[m] if m < 4 else attT[m - 4]
                nc.tensor.matmul(out_ps[:], lhsT=lhsT[:], rhs=Wc[m],
                                 start=(m == 0), stop=(m == 7))
            out_sb = smallp.tile([BROWS, C], f32, tag="out_sb")
            nc.scalar.copy(out=out_sb[:], in_=out_ps[:BROWS, :])
            nc.sync.dma_start(out=out_d[:, :], in_=out_sb[:])
    nc.finalize()
    return nc


def _phase1_nc():
    global _PH1
    if _PH1 is None:
        _PH1 = _build_phase1()
    return _PH1


def _phase2_nc():
    global _PH2
    if _PH2 is None:
        _PH2 = _build_phase2()
    return _PH2


def _quantize(query_feat, khat_pad):
    """Integer-quantize relu(q) and khat so that fp8e4 DoubleRow matmuls are
    exact and |sim| stays < 8192 (one fp32 binade under BIG)."""
    q32 = np.maximum(query_feat, 0)
    sq = 16.0 / q32.max()
    sk = 15.0 / np.abs(khat_pad).max()
    while True:
        q_int = np.rint(q32 * sq).astype(np.float32)            # 0..16
        k_int = 2.0 * np.rint(khat_pad * sk).astype(np.float32)  # even, |.|<=30
        qn = np.linalg.norm(q_int, axis=1).max()
        kn = np.linalg.norm(k_int, axis=1).max()
        if qn * kn < 8100.0:
            return q_int, k_int
        sq *= 0.95
        sk *= 0.97


def _knn_top32(query_feat, memory_keys):
    """Phase 1 on device + host merge: exact global top-32 indices [B, K]."""
    # ---- host prep: pad + normalize + quantize + rearrange keys ----
    kn = np.sqrt((memory_keys ** 2).sum(axis=1))
    khat = memory_keys * (1.0 / kn)[:, None]
    pad = np.full((NPAD - N, D), -1.0 / np.sqrt(D), np.float32)
    khat_pad = np.concatenate([khat.astype(np.float32), pad], axis=0)
    q_int, k_int = _quantize(query_feat, khat_pad)

    # kT arr: [core][25, 128, 2, 2, 512] <- k_int[c*12800 + 512c + n, dc*256+i*128+p]
    karr = k_int.reshape(NC_CORES, 25, 512, 2, 2, 128).transpose(0, 1, 5, 3, 4, 2)
    karr = np.ascontiguousarray(karr).astype(ml_dtypes.float8_e4m3)
    # qT arr: [2, 128, 2, 256] <- q_int[q, dc*256+i*128+p]
    qarr = q_int.T.reshape(2, 2, 128, B).transpose(2, 0, 1, 3)
    qarr = np.ascontiguousarray(qarr).astype(ml_dtypes.float8_e4m3)
    n = np.arange(WIN)
    bias = np.stack([np.full(WIN, BIG), (n // 32) * (32 * ULP), (n % 32) * ULP]
                    ).astype(np.float32)

    ph1 = _phase1_nc()
    in_maps = [{"kT": karr[c], "qT": qarr, "bias": bias} for c in range(NC_CORES)]
    res1 = run_bass_kernel_spmd(ph1, in_maps, core_ids=list(range(NC_CORES)))

    # ---- host: decode candidates, exact re-score, global top-32 ----
    cand_r = []   # row indices
    cand_k = []   # global key indices
    win_base = np.zeros(56, np.int64)       # l1 col -> window base (in-shard)
    win_base[0:8] = 6 * WIN
    for j in range(5):
        win_base[8 + 8 * j:16 + 8 * j] = (7 + j) * WIN
    win_base[48:56] = 12 * WIN
    rows128 = np.arange(128)
    for c in range(NC_CORES):
        l1 = res1.results[c]["l1"].view(np.uint32)      # [2, 128, 56]
        tk = res1.results[c]["tk"]                      # [2, NREG, 128, 32]
        for qt in range(2):
            # DVE path: packed low-10-bit in-window index
            ks = c * SHARD + win_base[None, :] + (l1[qt] & np.uint32(0x3FF))
            cand_k.append(ks.reshape(-1))
            cand_r.append(np.repeat(qt * 128 + rows128, 56))
            # topk path: flat idx within [16, TOPW] token slab
            for r in range(NREG):
                idx = tk[qt, r, :, 16:32].astype(np.int64).reshape(8, 256)
                p_rel = idx // TOPW
                col = idx % TOPW
                tok = np.arange(8)[:, None]
                rows = qt * 128 + tok * 16 + p_rel
                keys = c * SHARD + r * TOPW + col
                cand_r.append(rows.reshape(-1))
                cand_k.append(keys.reshape(-1))
    cand_r = np.concatenate(cand_r)
    cand_k = np.concatenate(cand_k)
    keep = cand_k < N
    cand_r = cand_r[keep]
    cand_k = cand_k[keep].astype(np.int64)

    # per-row candidate matrix (padded with key 0 dups; ordered by key index
    # for reference-stable tie-breaking)
    order = np.lexsort((cand_k, cand_r))
    cand_r = cand_r[order]
    cand_k = cand_k[order]
    counts = np.bincount(cand_r, minlength=B)
    maxc = int(counts.max())
    grid = np.zeros((B, maxc), np.int64)
    mask = np.zeros((B, maxc), bool)
    pos = (np.arange(cand_r.size) -
           np.concatenate([[0], np.cumsum(counts)[:-1]])[cand_r])
    grid[cand_r, pos] = cand_k
    mask[cand_r, pos] = True

    q32 = np.maximum(query_feat, 0)
    cand_keys = memory_keys[grid]                       # [B, maxc, D]
    dots = np.einsum("bd,bcd->bc", q32, cand_keys, optimize=True)
    cos = dots / np.maximum(
        np.linalg.norm(q32, axis=1)[:, None] * kn[grid], np.float32(1e-8))
    cos[~mask] = -np.inf
    # dedup: same key may arrive from both paths; keep first occurrence
    dup = np.zeros_like(mask)
    dup[:, 1:] = grid[:, 1:] == grid[:, :-1]
    cos[dup & mask] = -np.inf
    sel = np.argsort(-cos, axis=1, kind="stable")[:, :K]
    return np.take_along_axis(grid, sel, axis=1)        # [256, 32]


def kernel(query_feat, memory_keys, Wq, bq, Wm, bm, Ws, bs, Wc, bc):
    query_feat = np.asarray(query_feat, np.float32)
    memory_keys = np.asarray(memory_keys, np.float32)
    top_idx = _knn_top32(query_feat, memory_keys)
    knn = memory_keys[top_idx]                          # [256, 32, 512]

    # ---- phase 2 (batch sharded) ----
    ph2 = _phase2_nc()
    bqm = (np.asarray(bq, np.float32) + np.asarray(bm, np.float32)).reshape(A, 1)
    Wq_a = np.ascontiguousarray(np.asarray(Wq, np.float32))
    Wm_a = np.ascontiguousarray(np.asarray(Wm, np.float32))
    Ws_a = np.ascontiguousarray(np.asarray(Ws, np.float32))
    Wc_a = np.ascontiguousarray(np.asarray(Wc, np.float32))
    in_maps2 = []
    for c in range(NC_CORES):
        rows = slice(c * BROWS, (c + 1) * BROWS)
        knn_c = knn[rows].reshape(BROWS * K, D)
        in_maps2.append({
            "qT": np.ascontiguousarray(query_feat[rows].T),
            "knn": np.ascontiguousarray(knn_c),
            "knnT": np.ascontiguousarray(knn_c.T),
            "Wq": Wq_a, "Wm": Wm_a, "Ws": Ws_a, "bqm": bqm, "Wc": Wc_a,
        })
    res2 = run_bass_kernel_spmd(ph2, in_maps2, core_ids=list(range(NC_CORES)))
    out = np.concatenate([res2.results[c]["out"] for c in range(NC_CORES)], axis=0)
    return (out + np.asarray(bc, np.float32)[None, :]).astype(np.float32)


# revision 12
# speedup vs baseline: 1.6933x; 1.0601x over previous
"""Trainium2 Bass kernel for retrieval-knn attention classifier (nn_MA_51866025067137).

Strategy (8 NeuronCores):
  Phase 1 — memory_keys sharded along N (12800 keys/core, padded 100000->102400).
  Keys/queries are quantized to small integers and fed to fp8e4 DoubleRow
  matmuls (2 rows/cycle on the PE).  A 3-row fp32r "bias" matmul adds
  BIG + iota*2^-9 so every similarity lands in one fp32 binade [2^14, 2^15)
  with its low 10 mantissa bits equal to the column index (sims are exact
  even integers, so the pack costs nothing).  Per-1024 window top-8 is then a
  single DVE max8 straight out of PSUM; the first 6400 columns instead go
  through ACT eviction + two GPSIMD topk calls (top-256 per 16-partition
  token with indices).  Candidate extraction/merging/re-scoring is done on
  the host, which gathers the exact global top-32 key vectors.
  Phase 2 — batch sharded (32 queries/core): memory-attention module
  (tanh(qWq + knnWm + b) -> scores -> softmax -> weighted sum) and classifier,
  via small fp32r matmuls; the softmax-weighted sum is a block-diagonal matmul.
"""

import numpy as np
import ml_dtypes

import concourse.bacc as bacc
import concourse.mybir as mybir
from concourse.tile import TileContext, add_dep_helper
from concourse.bass_utils import run_bass_kernel_spmd
from concourse.masks import make_identity

# problem dims (hardcoded per harness contract)
B, N, D = 256, 100000, 512
A, C, K = 256, 100, 32
NC_CORES = 8
NPAD = 102400             # 8 * 12800
SHARD = NPAD // NC_CORES  # 12800
WIN = 1024                # DVE max8 window
NWIN = 13                 # windows 0..11 full, 12 is half (512)
TOPW = 3200               # gpsimd topk region width (vocab = 16*3200 = 51200)
NREG = 2                  # topk regions per qt: cols [0, 6400)
TOPC = NREG * TOPW        # 6400 cols to ACT-evict per qt
BROWS = B // NC_CORES     # 32 rows per core in phase 2
BIG = 24576.0             # binade [2^14, 2^15); ulp 2^-9
ULP = 2.0 ** -9

f32 = mybir.dt.float32
f32r = mybir.dt.float32r
f8 = mybir.dt.float8e4
u32 = mybir.dt.uint32

_PH1 = None
_PH2 = None


def _u(i):
    return i.ins if hasattr(i, "ins") else i


def _build_phase1():
    nc = bacc.Bacc("TRN2", target_bir_lowering=False)
    kT_d = nc.dram_tensor("kT", [13, 128, 2, 2, 2, 512], f8, kind="ExternalInput")
    qT_d = nc.dram_tensor("qT", [2, 128, 2, 2, 128], f8, kind="ExternalInput")
    l1_d = nc.dram_tensor("l1", [2, 128, 56], f32, kind="ExternalOutput")
    tk_d = nc.dram_tensor("tk", [2, NREG, 128, 32], u32, kind="ExternalOutput")

    with TileContext(nc) as tc:
        with (
            tc.tile_pool(name="const", bufs=1) as constp,
            tc.tile_pool(name="keys", bufs=6) as keyp,
            tc.tile_pool(name="l1", bufs=1) as l1p,
            tc.tile_pool(name="psum", bufs=2, space="PSUM") as psump,
        ):
            qT = [constp.tile([128, 2, 2, 128], f8, tag=f"qT{qt}", name=f"qT_t{qt}")
                  for qt in range(2)]
            for qt in range(2):
                nc.sync.dma_start(out=qT[qt][:], in_=qT_d[qt, :, :, :, :])

            # raw SBUF for gpsimd topk (per qt) + its output
            sims_sb = [[nc.alloc_sbuf_tensor(f"sims_sb{qt}_{r}", [128, TOPW], f32)
                        for r in range(NREG)] for qt in range(2)]
            tk_sb = [[nc.alloc_sbuf_tensor(f"tk_sb{qt}_{r}", [128, 32], u32)
                      for r in range(NREG)] for qt in range(2)]

            L1 = [l1p.tile([128, 56], f32, tag=f"l1_{qt}", name=f"l1_{qt}")
                  for qt in range(2)]
            evicts = [[[], []], [[], []]]  # [qt][region]: ACT evicts feeding topk

            for w in range(NWIN):
                wcols = 512 if w == 12 else WIN
                nchunk = wcols // 512
                kt = keyp.tile([128, 2, 2, 2, 512], f8, tag="kt", name="kt_t")
                nc.sync.dma_start(out=kt[:], in_=kT_d[w, :, :, :, :, :])
                for qt in range(2):
                    ps = psump.tile([128, WIN], f32, tag=f"win{qt}", name=f"ps{qt}")
                    for h in range(nchunk):
                        sl = slice(h * 512, (h + 1) * 512)
                        for dc in range(2):
                            nc.tensor.matmul(
                                ps[:, sl],
                                lhsT=qT[qt][:, dc, :, :],
                                rhs=kt[:, h, dc, :, :],
                                start=(dc == 0), stop=(dc == 1),
                                perf_mode=mybir.MatmulPerfMode.DoubleRow)
                    if w < 6:
                        # full window -> topk staging buffers (regions of 3200)
                        base = w * WIN
                        for r in range(NREG):
                            lo = max(base, r * TOPW)
                            hi = min(base + WIN, (r + 1) * TOPW)
                            if lo < hi:
                                ev = nc.scalar.copy(
                                    out=sims_sb[qt][r][:, lo - r * TOPW:hi - r * TOPW],
                                    in_=ps[:, lo - base:hi - base])
                                evicts[qt][r].append(ev)
                    elif w == 6:
                        ev = nc.scalar.copy(out=sims_sb[qt][1][:, 6 * WIN - TOPW:TOPW],
                                            in_=ps[:, :256])
                        evicts[qt][1].append(ev)
                        nc.vector.max(out=L1[qt][:, 0:8], in_=ps[:, 256:])
                    elif w < 12:
                        nc.vector.max(out=L1[qt][:, 8 * (w - 6):8 * (w - 5)], in_=ps[:])
                    else:
                        nc.vector.max(out=L1[qt][:, 48:56], in_=ps[:, :512])

            for qt in range(2):
                for r in range(NREG):
                    tki = nc.gpsimd.topk(
                        tk_sb[qt][r][:], sims_sb[qt][r][:],
                        tokens=8, vocab_size=16 * TOPW, k=256)
                    for ev in evicts[qt][r]:
                        add_dep_helper(_u(tki), _u(ev), reason="topk waits evicts")
                    do = nc.sync.dma_start(out=tk_d[qt, r, :, :],
                                           in_=tk_sb[qt][r][:])
                    add_dep_helper(_u(do), _u(tki), reason="tk out waits topk")
                nc.sync.dma_start(out=l1_d[qt, :, :], in_=L1[qt][:])
    nc.finalize()
    return nc


def _build_phase2():
    nc = bacc.Bacc("TRN2", target_bir_lowering=False)
    qT_in = nc.dram_tensor("qT", [D, BROWS], f32r, kind="ExternalInput")       # pre-relu
    knn_in = nc.dram_tensor("knn", [BROWS * K, D], f32r, kind="ExternalInput")
    knnT_in = nc.dram_tensor("knnT", [D, BROWS * K], f32r, kind="ExternalInput")
    Wq_in = nc.dram_tensor("Wq", [D, A], f32r, kind="ExternalInput")
    Wm_in = nc.dram_tensor("Wm", [D, A], f32r, kind="ExternalInput")
    Ws_in = nc.dram_tensor("Ws", [A, 1], f32r, kind="ExternalInput")
    bqm_in = nc.dram_tensor("bqm", [A, 1], f32, kind="ExternalInput")          # bq+bm
    Wc_in = nc.dram_tensor("Wc", [2 * D, C], f32r, kind="ExternalInput")
    out_d = nc.dram_tensor("out", [BROWS, C], f32, kind="ExternalOutput")      # +bc host
    escratch = nc.dram_tensor("escratch", [1, BROWS * K], f32)                 # bounce

    NCD = BROWS * K  # 1024

    with TileContext(nc) as tc:
        with (
            tc.tile_pool(name="big", bufs=1) as bigp,
            tc.tile_pool(name="small", bufs=1) as smallp,
            tc.tile_pool(name="psum", bufs=1, space="PSUM") as psump,
        ):
            # ---- load inputs (M-padded tiles to satisfy fp32r col_grp=0xf) ----
            qT = [smallp.tile([128, 128], f32r, tag=f"qT{dc}", name=f"qTt{dc}") for dc in range(4)]
            for dc in range(4):
                nc.vector.memset(qT[dc][:].bitcast(u32), 0)
                nc.sync.dma_start(out=qT[dc][:, :BROWS],
                                  in_=qT_in[dc * 128:(dc + 1) * 128, :])
                nc.scalar.activation(qT[dc][:, :BROWS], qT[dc][:, :BROWS],
                                     mybir.ActivationFunctionType.Relu)
            knnall = bigp.tile([128, 8 * D], f32r, tag="knnall")
            nc.sync.dma_start(out=knnall[:].rearrange("p (t d) -> p t d", t=8),
                              in_=knn_in[:].rearrange("(t p) d -> p t d", p=128))
            knn = [knnall[:, t * D:(t + 1) * D] for t in range(8)]
            knnTall = bigp.tile([128, 4 * NCD], f32r, tag="knnTall")
            nc.sync.dma_start(out=knnTall[:].rearrange("p (dc c) -> p dc c", dc=4),
                              in_=knnT_in[:].rearrange("(dc p) c -> p dc c", p=128))
            knnT = [knnTall[:, dc * NCD:(dc + 1) * NCD] for dc in range(4)]
            Wqall = smallp.tile([128, 4 * A], f32r, tag="Wqall")
            nc.sync.dma_start(out=Wqall[:].rearrange("p (dc a) -> p dc a", dc=4),
                              in_=Wq_in[:].rearrange("(dc p) a -> p dc a", p=128))
            Wq = [Wqall[:, dc * A:(dc + 1) * A] for dc in range(4)]
            Wmall = smallp.tile([128, 4 * A], f32r, tag="Wmall")
            nc.sync.dma_start(out=Wmall[:].rearrange("p (dc a) -> p dc a", dc=4),
                              in_=Wm_in[:].rearrange("(dc p) a -> p dc a", p=128))
            Wm = [Wmall[:, dc * A:(dc + 1) * A] for dc in range(4)]
            Ws = [smallp.tile([128, 128], f32r, tag=f"Ws{at}", name=f"Wst{at}") for at in range(2)]
            bqm = [smallp.tile([128, 1], f32, tag=f"bqm{at}", name=f"bqmt{at}") for at in range(2)]
            for at in range(2):
                nc.vector.memset(Ws[at][:].bitcast(u32), 0)
                nc.sync.dma_start(out=Ws[at][:, :1],
                                  in_=Ws_in[at * 128:(at + 1) * 128, :])
                nc.sync.dma_start(out=bqm[at][:],
                                  in_=bqm_in[at * 128:(at + 1) * 128, :])
            Wcall = smallp.tile([128, 8 * C], f32r, tag="Wcall")
            nc.sync.dma_start(out=Wcall[:].rearrange("p (m j) -> p m j", m=8),
                              in_=Wc_in[:].rearrange("(m p) j -> p m j", p=128))
            Wc = [Wcall[:, m * C:(m + 1) * C] for m in range(8)]
            ones = smallp.tile([128, 2], f32r, tag="ones")
            nc.vector.memset(ones[:].bitcast(u32), 0)
            nc.vector.memset(ones[:, :1].bitcast(u32), 0x3F800000)
            # mask4[p, j] = 1.0 iff j == p // 32
            mask4 = smallp.tile([128, 4], f32, tag="mask4")
            nc.vector.memset(mask4[:], 1.0)
            nc.gpsimd.affine_select(out=mask4[:], in_=mask4[:],
                                    compare_op=mybir.AluOpType.is_ge, fill=0.0,
                                    base=0, pattern=[[-32, 4]], channel_multiplier=1)
            nc.gpsimd.affine_select(out=mask4[:], in_=mask4[:],
                                    compare_op=mybir.AluOpType.is_ge, fill=0.0,
                                    base=31, pattern=[[32, 4]], channel_multiplier=-1)
            ident = smallp.tile([128, 128], f32, tag="ident")
            make_identity(nc, ident[:])

            # ---- qprojT [2][128a, 32] ----
            qprojT = [smallp.tile([128, BROWS], f32, tag=f"qp{at}", name=f"qpt{at}") for at in range(2)]
            for at in range(2):
                ps = psump.tile([128, BROWS], f32, tag="ps_a")
                for dc in range(4):
                    nc.tensor.matmul(
                        ps[:],
                        lhsT=Wq[dc][:, at * 128:(at + 1) * 128],
                        rhs=qT[dc][:, :BROWS],
                        start=(dc == 0), stop=(dc == 3))
                nc.scalar.copy(out=qprojT[at][:], in_=ps[:])

            # ---- hT = tanh(kprojT + qprojT_bcast + bqm) ; scores ----
            sc_ps = psump.tile([128, NCD], f32, tag="ps_sc")
            for at in range(2):
                kp = psump.tile([128, NCD], f32, tag="ps_kp", bufs=2)
                for dc in range(4):
                    for half in range(2):
                        nc.tensor.matmul(
                            kp[:, half * 512:(half + 1) * 512],
                            lhsT=Wm[dc][:, at * 128:(at + 1) * 128],
                            rhs=knnT[dc][:, half * 512:(half + 1) * 512],
                            start=(dc == 0), stop=(dc == 3))
                hT = bigp.tile([128, NCD], f32r, tag=f"hT{at}")
                qb = qprojT[at][:, :, None].to_broadcast([128, BROWS, K])
                nc.vector.tensor_tensor(
                    hT[:].rearrange("p (q k) -> p q k", k=K),
                    kp[:].rearrange("p (q k) -> p q k", k=K),
                    qb, mybir.AluOpType.add)
                nc.scalar.activation(hT[:], hT[:], mybir.ActivationFunctionType.Tanh,
                                     bias=bqm[at][:])
                for half in range(2):
                    nc.tensor.matmul(
                        sc_ps[:, half * 512:(half + 1) * 512],
                        lhsT=Ws[at][:],
                        rhs=hT[:, half * 512:(half + 1) * 512],
                        start=(at == 0), stop=(at == 1))
            e_row = smallp.tile([1, NCD], f32, tag="e_row")
            nc.scalar.activation(e_row[:], sc_ps[:1, :],
                                 mybir.ActivationFunctionType.Exp)
            # bounce through DRAM to redistribute [1, 1024] -> [128, 8]
            nc.sync.dma_start(out=escratch[:, :], in_=e_row[:, :])
            e_col = smallp.tile([128, 8], f32, tag="e_col")
            nc.sync.dma_start(out=e_col[:],
                              in_=escratch[0, :].rearrange("(t p) -> p t", p=128))

            # ---- block-diag softmax weights (M-padded), den, attended ----
            w2 = [bigp.tile([128, 128], f32r, tag=f"w2_{t}", name=f"w2t{t}") for t in range(8)]
            for t in range(8):
                nc.vector.memset(w2[t][:].bitcast(u32), 0)
                nc.vector.tensor_scalar_mul(w2[t][:, 4 * t:4 * t + 4], mask4[:],
                                            e_col[:, t:t + 1])
            den_ps = psump.tile([128, 2], f32, tag="ps_a")
            for t in range(8):
                nc.tensor.matmul(den_ps[:], lhsT=w2[t][:], rhs=ones[:],
                                 start=(t == 0), stop=(t == 7))
            att_ps = psump.tile([128, D], f32, tag="ps_kp", bufs=2)
            for t in range(8):
                nc.tensor.matmul(att_ps[:], lhsT=w2[t][:], rhs=knn[t],
                                 start=(t == 0), stop=(t == 7))
            rden = smallp.tile([BROWS, 1], f32, tag="rden")
            nc.vector.reciprocal(rden[:], den_ps[:BROWS, :1])
            att = smallp.tile([BROWS, D], f32, tag="att_sb")
            nc.vector.tensor_scalar_mul(att[:], att_ps[:BROWS, :], rden[:])

            # ---- attendedT via PE transpose (plain fp32) ----
            attT = [smallp.tile([128, 128], f32r, tag=f"attT{dc}", name=f"attTt{dc}") for dc in range(4)]
            for dc in range(4):
                tp = psump.tile([128, BROWS], f32, tag="ps_a")
                nc.tensor.transpose(tp[:], att[:, dc * 128:(dc + 1) * 128],
                                    ident[:BROWS, :BROWS])
                nc.vector.memset(attT[dc][:].bitcast(u32), 0)
                nc.scalar.copy(out=attT[dc][:, :BROWS], in_=tp[:])

            # ---- classifier ----
            out_ps = psump.tile([128, C], f32, tag="ps_out")
            for m in range(8):
                lhsT = qT[m] if m < 4 else attT[m - 4]
                nc.tensor.matmul(out_ps[:], lhsT=lhsT[:], rhs=Wc[m],
                                 start=(m == 0), stop=(m == 7))
            out_sb = smallp.tile([BROWS, C], f32, tag="out_sb")
            nc.scalar.copy(out=out_sb[:], in_=out_ps[:BROWS, :])
            nc.sync.dma_start(out=out_d[:, :], in_=out_sb[:])
    nc.finalize()
    return nc


def _phase1_nc():
    global _PH1
    if _PH1 is None:
        _PH1 = _build_phase1()
    return _PH1


def _phase2_nc():
    global _PH2
    if _PH2 is None:
        _PH2 = _build_phase2()
    return _PH2


def _quantize(query_feat, khat_pad):
    """Integer-quantize relu(q) and khat so that fp8e4 DoubleRow matmuls are
    exact and |sim| stays < 8192 (one fp32 binade under BIG)."""
    q32 = np.maximum(query_feat, 0)
    sq = 16.0 / q32.max()
    sk = 15.0 / np.abs(khat_pad).max()
    while True:
        q_int = np.rint(q32 * sq).astype(np.float32)            # 0..16
        k_int = 2.0 * np.rint(khat_pad * sk).astype(np.float32)  # even, |.|<=30
        qn = np.linalg.norm(q_int, axis=1).max()
        kn = np.linalg.norm(k_int, axis=1).max()
        if qn * kn < 8100.0:
            return q_int, k_int
        sq *= 0.95
        sk *= 0.97


def _knn_top32(query_feat, memory_keys):
    """Phase 1 on device + host merge: exact global top-32 indices [B, K]."""
    # ---- host prep: pad + normalize + quantize + rearrange keys ----
    kn = np.sqrt((memory_keys ** 2).sum(axis=1))
    khat = memory_keys * (1.0 / kn)[:, None]
    pad = np.full((NPAD - N, D), -1.0 / np.sqrt(D), np.float32)
    khat_pad = np.concatenate([khat.astype(np.float32), pad], axis=0)
    q_int, k_int = _quantize(query_feat, khat_pad)
    # dims {381..383, 509..511} are repurposed as bias rows: value =
    # BIG + nw*2^-9 where nw = in-window column (0..1023).  The PE sums each
    # DoubleRow (i=0,i=1) pair in ~fp16 before fp32 PSUM, so each pair-sum
    # must be fp16-exact: BIG alone, (a,b) together, c alone.
    q_int[:, [381, 382, 383, 509, 510, 511]] = 0.0
    k_int[:, [381, 382, 383, 509, 510, 511]] = 0.0

    # kT arr: [core][13, 128, 2(h), 2(dc), 2(i), 512(n)]
    #   <- k_int[c*12800 + (2w+h)*512 + n, dc*256 + i*128 + p]; chunk 26 = pad
    k_ext = np.concatenate(
        [k_int.reshape(NC_CORES, 25, 512, D),
         np.full((NC_CORES, 1, 512, D), -30.0, np.float32)], axis=1)
    karr = k_ext.reshape(NC_CORES, 13, 2, 512, 2, 2, 128).transpose(0, 1, 6, 2, 4, 5, 3)
    karr = np.ascontiguousarray(karr)                   # [c, w, p, h, dc, i, n]
    nw = (np.arange(2)[:, None] * 512 + np.arange(512)[None, :]).astype(np.float32)
    karr[:, :, 125, :, 1, 0, :] = 192.0                 # q 128     -> +24576
    karr[:, :, 125, :, 1, 1, :] = 0.0
    karr[:, :, 126, :, 1, 0, :] = np.floor(nw / 256)    # q 2^-1    -> a*2^-1
    karr[:, :, 126, :, 1, 1, :] = np.floor(nw / 16) % 16  # q 2^-5  -> b*2^-5
    karr[:, :, 127, :, 1, 0, :] = nw % 16               # q 2^-9    -> c*2^-9
    karr[:, :, 127, :, 1, 1, :] = 0.0
    karr = karr.astype(ml_dtypes.float8_e4m3)
    # qT arr: [2, 128, 2, 256] <- q_int[q, dc*256+i*128+p]
    qarr = q_int.T.reshape(2, 2, 128, 2, 128).transpose(3, 2, 0, 1, 4)
    qarr = np.ascontiguousarray(qarr)                   # [qt, p, dc, i, q]
    qarr[:, 125, 1, 0, :] = 128.0
    qarr[:, 125, 1, 1, :] = 0.0
    qarr[:, 126, 1, 0, :] = 0.5
    qarr[:, 126, 1, 1, :] = 2.0 ** -5
    qarr[:, 127, 1, 0, :] = 2.0 ** -9
    qarr[:, 127, 1, 1, :] = 0.0
    qarr = qarr.astype(ml_dtypes.float8_e4m3)

    ph1 = _phase1_nc()
    in_maps = [{"kT": karr[c], "qT": qarr} for c in range(NC_CORES)]
    res1 = run_bass_kernel_spmd(ph1, in_maps, core_ids=list(range(NC_CORES)))

    # ---- host: decode candidates, exact re-score, global top-32 ----
    cand_r = []   # row indices
    cand_k = []   # global key indices
    win_base = np.zeros(56, np.int64)       # l1 col -> window base (in-shard)
    win_base[0:8] = 6 * WIN
    for j in range(5):
        win_base[8 + 8 * j:16 + 8 * j] = (7 + j) * WIN
    win_base[48:56] = 12 * WIN
    rows128 = np.arange(128)
    for c in range(NC_CORES):
        l1 = res1.results[c]["l1"].view(np.uint32)      # [2, 128, 56]
        tk = res1.results[c]["tk"]                      # [2, NREG, 128, 32]
        for qt in range(2):
            # DVE path: packed low-10-bit in-window index
            ks = c * SHARD + win_base[None, :] + (l1[qt] & np.uint32(0x3FF))
            cand_k.append(ks.reshape(-1))
            cand_r.append(np.repeat(qt * 128 + rows128, 56))
            # topk path: flat idx within [16, TOPW] token slab
            for r in range(NREG):
                idx = tk[qt, r, :, 16:32].astype(np.int64).reshape(8, 256)
                p_rel = idx // TOPW
                col = idx % TOPW
                tok = np.arange(8)[:, None]
                rows = qt * 128 + tok * 16 + p_rel
                keys = c * SHARD + r * TOPW + col
                cand_r.append(rows.reshape(-1))
                cand_k.append(keys.reshape(-1))
    cand_r = np.concatenate(cand_r)
    cand_k = np.concatenate(cand_k)
    keep = cand_k < N
    cand_r = cand_r[keep]
    cand_k = cand_k[keep].astype(np.int64)

    # per-row candidate matrix (padded with key 0 dups; ordered by key index
    # for reference-stable tie-breaking)
    order = np.lexsort((cand_k, cand_r))
    cand_r = cand_r[order]
    cand_k = cand_k[order]
    counts = np.bincount(cand_r, minlength=B)
    maxc = int(counts.max())
    grid = np.zeros((B, maxc), np.int64)
    mask = np.zeros((B, maxc), bool)
    pos = (np.arange(cand_r.size) -
           np.concatenate([[0], np.cumsum(counts)[:-1]])[cand_r])
    grid[cand_r, pos] = cand_k
    mask[cand_r, pos] = True

    q32 = np.maximum(query_feat, 0)
    cand_keys = memory_keys[grid]                       # [B, maxc, D]
    dots = np.einsum("bd,bcd->bc", q32, cand_keys, optimize=True)
    cos = dots / np.maximum(
        np.linalg.norm(q32, axis=1)[:, None] * kn[grid], np.float32(1e-8))
    cos[~mask] = -np.inf
    # dedup: same key may arrive from both paths; keep first occurrence
    dup = np.zeros_like(mask)
    dup[:, 1:] = grid[:, 1:] == grid[:, :-1]
    cos[dup & mask] = -np.inf
    sel = np.argsort(-cos, axis=1, kind="stable")[:, :K]
    return np.take_along_axis(grid, sel, axis=1)        # [256, 32]


def kernel(query_feat, memory_keys, Wq, bq, Wm, bm, Ws, bs, Wc, bc):
    query_feat = np.asarray(query_feat, np.float32)
    memory_keys = np.asarray(memory_keys, np.float32)
    top_idx = _knn_top32(query_feat, memory_keys)
    knn = memory_keys[top_idx]                          # [256, 32, 512]

    # ---- phase 2 (batch sharded) ----
    ph2 = _phase2_nc()
    bqm = (np.asarray(bq, np.float32) + np.asarray(bm, np.float32)).reshape(A, 1)
    Wq_a = np.ascontiguousarray(np.asarray(Wq, np.float32))
    Wm_a = np.ascontiguousarray(np.asarray(Wm, np.float32))
    Ws_a = np.ascontiguousarray(np.asarray(Ws, np.float32))
    Wc_a = np.ascontiguousarray(np.asarray(Wc, np.float32))
    in_maps2 = []
    for c in range(NC_CORES):
        rows = slice(c * BROWS, (c + 1) * BROWS)
        knn_c = knn[rows].reshape(BROWS * K, D)
        in_maps2.append({
            "qT": np.ascontiguousarray(query_feat[rows].T),
            "knn": np.ascontiguousarray(knn_c),
            "knnT": np.ascontiguousarray(knn_c.T),
            "Wq": Wq_a, "Wm": Wm_a, "Ws": Ws_a, "bqm": bqm, "Wc": Wc_a,
        })
    res2 = run_bass_kernel_spmd(ph2, in_maps2, core_ids=list(range(NC_CORES)))
    out = np.concatenate([res2.results[c]["out"] for c in range(NC_CORES)], axis=0)
    return (out + np.asarray(bc, np.float32)[None, :]).astype(np.float32)


# revision 16
# speedup vs baseline: 2.3437x; 1.3841x over previous
"""Trainium2 Bass kernel for retrieval-knn attention classifier (nn_MA_51866025067137).

Strategy (8 NeuronCores):
  Phase 1 — memory_keys sharded along N (12800 keys/core, padded 100000->102400).
  Keys/queries are quantized to small integers and fed to fp8e4 DoubleRow
  matmuls (2 rows/cycle on the PE).  A 3-row fp32r "bias" matmul adds
  BIG + iota*2^-9 so every similarity lands in one fp32 binade [2^14, 2^15)
  with its low 10 mantissa bits equal to the column index (sims are exact
  even integers, so the pack costs nothing).  Per-1024 window top-8 is then a
  single DVE max8 straight out of PSUM; the first 6400 columns instead go
  through ACT eviction + two GPSIMD topk calls (top-256 per 16-partition
  token with indices).  Candidate extraction/merging/re-scoring is done on
  the host, which gathers the exact global top-32 key vectors.
  Phase 2 — batch sharded (32 queries/core): memory-attention module
  (tanh(qWq + knnWm + b) -> scores -> softmax -> weighted sum) and classifier,
  via small fp32r matmuls; the softmax-weighted sum is a block-diagonal matmul.
"""

import numpy as np
import ml_dtypes

import concourse.bacc as bacc
import concourse.mybir as mybir
from concourse.tile import TileContext, add_dep_helper
from concourse.bass_utils import run_bass_kernel_spmd
from concourse.masks import make_identity

# problem dims (hardcoded per harness contract)
B, N, D = 256, 100000, 512
A, C, K = 256, 100, 32
NC_CORES = 8
NPAD = 102400             # 8 * 12800
SHARD = NPAD // NC_CORES  # 12800
WIN = 1024                # DVE max8 window
NWIN = 13                 # windows 0..11 full, 12 is half (512)
TOPW = 3200               # gpsimd topk region width (vocab = 16*3200 = 51200)
NREG = 2                  # topk regions per qt: cols [0, 6400)
TOPC = NREG * TOPW        # 6400 cols to ACT-evict per qt
BROWS = B // NC_CORES     # 32 rows per core in phase 2
BIG = 24576.0             # binade [2^14, 2^15); ulp 2^-9
ULP = 2.0 ** -9

f32 = mybir.dt.float32
f32r = mybir.dt.float32r
f8 = mybir.dt.float8e4
u32 = mybir.dt.uint32
bf16 = mybir.dt.bfloat16

_PH1 = None
_PH2 = None


def _u(i):
    return i.ins if hasattr(i, "ins") else i


def _build_phase1():
    nc = bacc.Bacc("TRN2", target_bir_lowering=False)
    kT_d = nc.dram_tensor("kT", [13, 128, 2, 2, 2, 512], f8, kind="ExternalInput")
    qT_d = nc.dram_tensor("qT", [2, 128, 2, 2, 128], f8, kind="ExternalInput")
    l1_d = nc.dram_tensor("l1", [2, 128, 56], f32, kind="ExternalOutput")
    tk_d = nc.dram_tensor("tk", [2, NREG, 128, 32], u32, kind="ExternalOutput")

    with TileContext(nc) as tc:
        with (
            tc.tile_pool(name="const", bufs=1) as constp,
            tc.tile_pool(name="keys", bufs=6) as keyp,
            tc.tile_pool(name="l1", bufs=1) as l1p,
            tc.tile_pool(name="psum", bufs=2, space="PSUM") as psump,
        ):
            qT = [constp.tile([128, 2, 2, 128], f8, tag=f"qT{qt}", name=f"qT_t{qt}")
                  for qt in range(2)]
            for qt in range(2):
                nc.sync.dma_start(out=qT[qt][:], in_=qT_d[qt, :, :, :, :])

            # raw SBUF for gpsimd topk (per qt) + its output
            sims_sb = [[nc.alloc_sbuf_tensor(f"sims_sb{qt}_{r}", [128, TOPW], f32)
                        for r in range(NREG)] for qt in range(2)]
            tk_sb = [[nc.alloc_sbuf_tensor(f"tk_sb{qt}_{r}", [128, 32], u32)
                      for r in range(NREG)] for qt in range(2)]

            L1 = [l1p.tile([128, 56], f32, tag=f"l1_{qt}", name=f"l1_{qt}")
                  for qt in range(2)]
            evicts = [[[], []], [[], []]]  # [qt][region]: ACT evicts feeding topk

            for w in range(NWIN):
                wcols = 512 if w == 12 else WIN
                nchunk = wcols // 512
                kt = keyp.tile([128, 2, 2, 2, 512], f8, tag="kt", name="kt_t")
                nc.sync.dma_start(out=kt[:], in_=kT_d[w, :, :, :, :, :])
                for qt in range(2):
                    ps = psump.tile([128, WIN], f32, tag=f"win{qt}", name=f"ps{qt}")
                    for h in range(nchunk):
                        sl = slice(h * 512, (h + 1) * 512)
                        for dc in range(2):
                            nc.tensor.matmul(
                                ps[:, sl],
                                lhsT=qT[qt][:, dc, :, :],
                                rhs=kt[:, h, dc, :, :],
                                start=(dc == 0), stop=(dc == 1),
                                perf_mode=mybir.MatmulPerfMode.DoubleRow)
                    if w < 6:
                        # full window -> topk staging buffers (regions of 3200)
                        base = w * WIN
                        for r in range(NREG):
                            lo = max(base, r * TOPW)
                            hi = min(base + WIN, (r + 1) * TOPW)
                            if lo < hi:
                                ev = nc.scalar.copy(
                                    out=sims_sb[qt][r][:, lo - r * TOPW:hi - r * TOPW],
                                    in_=ps[:, lo - base:hi - base])
                                evicts[qt][r].append(ev)
                    elif w == 6:
                        ev = nc.scalar.copy(out=sims_sb[qt][1][:, 6 * WIN - TOPW:TOPW],
                                            in_=ps[:, :256])
                        evicts[qt][1].append(ev)
                        nc.vector.max(out=L1[qt][:, 0:8], in_=ps[:, 256:])
                    elif w < 12:
                        nc.vector.max(out=L1[qt][:, 8 * (w - 6):8 * (w - 5)], in_=ps[:])
                    else:
                        nc.vector.max(out=L1[qt][:, 48:56], in_=ps[:, :512])

            for qt in range(2):
                for r in range(NREG):
                    tki = nc.gpsimd.topk(
                        tk_sb[qt][r][:], sims_sb[qt][r][:],
                        tokens=8, vocab_size=16 * TOPW, k=256)
                    for ev in evicts[qt][r]:
                        add_dep_helper(_u(tki), _u(ev), reason="topk waits evicts")
                    do = nc.sync.dma_start(out=tk_d[qt, r, :, :],
                                           in_=tk_sb[qt][r][:])
                    add_dep_helper(_u(do), _u(tki), reason="tk out waits topk")
                nc.sync.dma_start(out=l1_d[qt, :, :], in_=L1[qt][:])
    nc.finalize()
    return nc


def _build_phase2():
    nc = bacc.Bacc("TRN2", target_bir_lowering=False)
    NCD = BROWS * K  # 1024
    qTr_in = nc.dram_tensor("qTr", [D, BROWS], bf16, kind="ExternalInput")    # relu'd
    knnT_in = nc.dram_tensor("knnT", [D, NCD], bf16, kind="ExternalInput")
    knn_in = nc.dram_tensor("knn", [NCD, D], bf16, kind="ExternalInput")
    Wq_in = nc.dram_tensor("Wq", [D, A], bf16, kind="ExternalInput")
    Wm_in = nc.dram_tensor("Wm", [D, A], bf16, kind="ExternalInput")
    Ws_in = nc.dram_tensor("Ws", [A, 1], bf16, kind="ExternalInput")
    bqm_in = nc.dram_tensor("bqm", [A, 1], f32, kind="ExternalInput")         # bq+bm
    Wc_in = nc.dram_tensor("Wc", [2 * D, C], bf16, kind="ExternalInput")
    S_in = nc.dram_tensor("S", [BROWS, NCD], bf16, kind="ExternalInput")      # S[b,(b',k)]=d_bb'
    m256_in = nc.dram_tensor("m256", [128, 256], bf16, kind="ExternalInput")
    out_d = nc.dram_tensor("out", [BROWS, C], f32, kind="ExternalOutput")     # +bc host

    with TileContext(nc) as tc:
        with (
            tc.tile_pool(name="big", bufs=1) as bigp,
            tc.tile_pool(name="small", bufs=1) as smallp,
            tc.tile_pool(name="psum", bufs=1, space="PSUM") as psump,
        ):
            # ---- loads, ordered for earliest compute start ----
            qTr = smallp.tile([128, 4, BROWS], bf16, tag="qTr")
            nc.sync.dma_start(out=qTr[:],
                              in_=qTr_in[:].rearrange("(dc p) b -> p dc b", p=128))
            Wqall = smallp.tile([128, 4, A], bf16, tag="Wqall")
            nc.sync.dma_start(out=Wqall[:],
                              in_=Wq_in[:].rearrange("(dc p) a -> p dc a", p=128))
            knnTall = bigp.tile([128, 4, NCD], bf16, tag="knnTall")
            for dc in range(4):
                nc.sync.dma_start(out=knnTall[:, dc, :],
                                  in_=knnT_in[dc * 128:(dc + 1) * 128, :])
            Wmall = smallp.tile([128, 4, A], bf16, tag="Wmall")
            nc.sync.dma_start(out=Wmall[:],
                              in_=Wm_in[:].rearrange("(dc p) a -> p dc a", p=128))
            S = smallp.tile([BROWS, NCD], bf16, tag="S")
            nc.sync.dma_start(out=S[:], in_=S_in[:, :])
            Ws = [smallp.tile([128, 1], bf16, tag=f"Ws{at}", name=f"Wst{at}")
                  for at in range(2)]
            bqm = [smallp.tile([128, 1], f32, tag=f"bqm{at}", name=f"bqmt{at}")
                   for at in range(2)]
            for at in range(2):
                nc.sync.dma_start(out=Ws[at][:], in_=Ws_in[at * 128:(at + 1) * 128, :])
                nc.sync.dma_start(out=bqm[at][:], in_=bqm_in[at * 128:(at + 1) * 128, :])
            m256 = smallp.tile([128, 256], bf16, tag="m256")
            nc.sync.dma_start(out=m256[:], in_=m256_in[:, :])
            knnall = bigp.tile([128, 8, D], bf16, tag="knnall")
            nc.sync.dma_start(out=knnall[:],
                              in_=knn_in[:].rearrange("(t p) d -> p t d", p=128))
            Wcall = smallp.tile([128, 8, C], bf16, tag="Wcall")
            nc.sync.dma_start(out=Wcall[:],
                              in_=Wc_in[:].rearrange("(m p) j -> p m j", p=128))
            ones = smallp.tile([128, 1], bf16, tag="ones")
            nc.vector.memset(ones[:].bitcast(mybir.dt.uint16), 0x3F80)
            ident = smallp.tile([128, 128], f32, tag="ident")
            make_identity(nc, ident[:])

            # ---- qproj [32b, 256a] = qTr.T @ Wq ----
            qp_ps = psump.tile([BROWS, A], f32, tag="ps_misc", bufs=2, name="qp_ps")
            for dc in range(4):
                nc.tensor.matmul(qp_ps[:], lhsT=qTr[:, dc, :], rhs=Wqall[:, dc, :],
                                 start=(dc == 0), stop=(dc == 3))
            qproj = smallp.tile([BROWS, A], bf16, tag="qproj")
            nc.scalar.copy(out=qproj[:], in_=qp_ps[:])

            # ---- h^T = tanh(Wm.T knn.T + qproj via S + bqm); scores ----
            sc_ps = psump.tile([1, NCD], f32, tag="ps_sc")
            hT = [bigp.tile([128, NCD], bf16, tag=f"hT{at}", name=f"hTt{at}")
                  for at in range(2)]
            for at in range(2):
                kp = psump.tile([128, NCD], f32, tag="ps_kp", bufs=2)
                for dc in range(4):
                    for half in range(2):
                        nc.tensor.matmul(
                            kp[:, half * 512:(half + 1) * 512],
                            lhsT=Wmall[:, dc, at * 128:(at + 1) * 128],
                            rhs=knnTall[:, dc, half * 512:(half + 1) * 512],
                            start=(dc == 0), stop=False)
                for half in range(2):
                    nc.tensor.matmul(
                        kp[:, half * 512:(half + 1) * 512],
                        lhsT=qproj[:, at * 128:(at + 1) * 128],
                        rhs=S[:, half * 512:(half + 1) * 512],
                        start=False, stop=(half == 1))
                nc.scalar.activation(hT[at][:], kp[:],
                                     mybir.ActivationFunctionType.Tanh,
                                     bias=bqm[at][:])
                for half in range(2):
                    nc.tensor.matmul(
                        sc_ps[:, half * 512:(half + 1) * 512],
                        lhsT=Ws[at][:],
                        rhs=hT[at][:, half * 512:(half + 1) * 512],
                        start=(at == 0), stop=(at == 1))
            e_row = smallp.tile([1, NCD], f32, tag="e_row")
            nc.scalar.activation(e_row[:], sc_ps[:1, :],
                                 mybir.ActivationFunctionType.Exp)
            # [1, 1024] -> [128, 8] via 8 PE transposes
            ec_ps = psump.tile([128, 8], f32, tag="ps_misc", bufs=2, name="ec_ps")
            for t in range(8):
                nc.tensor.transpose(ec_ps[:, t:t + 1], e_row[:1, t * 128:(t + 1) * 128],
                                    ident[:1, :1])

            # ---- w2[p, (t,j)] = e_col[p, t] * m256 ; den; attT ----
            w2 = bigp.tile([128, 256], bf16, tag="w2")
            eb = ec_ps[:, :, None].to_broadcast([128, 8, 32])
            nc.vector.tensor_tensor(w2[:].rearrange("p (t j) -> p t j", t=8),
                                    m256[:].rearrange("p (t j) -> p t j", t=8),
                                    eb, mybir.AluOpType.mult)
            den_ps = psump.tile([BROWS, 1], f32, tag="ps_misc", bufs=2, name="den_ps")
            for t in range(8):
                nc.tensor.matmul(den_ps[:], lhsT=w2[:, t * 32:(t + 1) * 32],
                                 rhs=ones[:], start=(t == 0), stop=(t == 7))
            rden = smallp.tile([BROWS, 1], f32, tag="rden")
            nc.vector.reciprocal(rden[:], den_ps[:])
            attT = smallp.tile([128, 4, BROWS], bf16, tag="attT")
            for dc in range(4):
                att_ps = psump.tile([128, BROWS], f32, tag="ps_misc", bufs=2,
                                    name=f"att_ps{dc}")
                for t in range(8):
                    nc.tensor.matmul(att_ps[:],
                                     lhsT=knnall[:, t, dc * 128:(dc + 1) * 128],
                                     rhs=w2[:, t * 32:(t + 1) * 32],
                                     start=(t == 0), stop=(t == 7))
                nc.scalar.copy(out=attT[:, dc, :], in_=att_ps[:])

            # ---- classifier: out = q-part + rden * att-part (att unnormalized) ----
            out1 = psump.tile([BROWS, C], f32, tag="ps_misc", bufs=2, name="out1")
            out2 = psump.tile([BROWS, C], f32, tag="ps_misc", bufs=2, name="out2")
            for dc in range(4):
                nc.tensor.matmul(out1[:], lhsT=qTr[:, dc, :], rhs=Wcall[:, dc, :],
                                 start=(dc == 0), stop=(dc == 3))
                nc.tensor.matmul(out2[:], lhsT=attT[:, dc, :], rhs=Wcall[:, 4 + dc, :],
                                 start=(dc == 0), stop=(dc == 3))
            out1_sb = smallp.tile([BROWS, C], f32, tag="out1_sb")
            nc.scalar.copy(out=out1_sb[:], in_=out1[:])
            out_sb = smallp.tile([BROWS, C], f32, tag="out_sb")
            nc.vector.scalar_tensor_tensor(
                out=out_sb[:], in0=out2[:], scalar=rden[:], in1=out1_sb[:],
                op0=mybir.AluOpType.mult, op1=mybir.AluOpType.add)
            nc.sync.dma_start(out=out_d[:, :], in_=out_sb[:])
    nc.finalize()
    return nc


def _phase1_nc():
    global _PH1
    if _PH1 is None:
        _PH1 = _build_phase1()
    return _PH1


def _phase2_nc():
    global _PH2
    if _PH2 is None:
        _PH2 = _build_phase2()
    return _PH2


def _quantize(query_feat, khat_pad):
    """Integer-quantize relu(q) and khat so that fp8e4 DoubleRow matmuls are
    exact and |sim| stays < 8192 (one fp32 binade under BIG)."""
    q32 = np.maximum(query_feat, 0)
    sq = 16.0 / q32.max()
    sk = 15.0 / np.abs(khat_pad).max()
    while True:
        q_int = np.rint(q32 * sq).astype(np.float32)            # 0..16
        k_int = 2.0 * np.rint(khat_pad * sk).astype(np.float32)  # even, |.|<=30
        qn = np.linalg.norm(q_int, axis=1).max()
        kn = np.linalg.norm(k_int, axis=1).max()
        if qn * kn < 8100.0:
            return q_int, k_int
        sq *= 0.95
        sk *= 0.97


def _knn_top32(query_feat, memory_keys):
    """Phase 1 on device + host merge: exact global top-32 indices [B, K]."""
    # ---- host prep: pad + normalize + quantize + rearrange keys ----
    kn = np.sqrt((memory_keys ** 2).sum(axis=1))
    khat = memory_keys * (1.0 / kn)[:, None]
    pad = np.full((NPAD - N, D), -1.0 / np.sqrt(D), np.float32)
    khat_pad = np.concatenate([khat.astype(np.float32), pad], axis=0)
    q_int, k_int = _quantize(query_feat, khat_pad)
    # dims {381..383, 509..511} are repurposed as bias rows: value =
    # BIG + nw*2^-9 where nw = in-window column (0..1023).  The PE sums each
    # DoubleRow (i=0,i=1) pair in ~fp16 before fp32 PSUM, so each pair-sum
    # must be fp16-exact: BIG alone, (a,b) together, c alone.
    q_int[:, [381, 382, 383, 509, 510, 511]] = 0.0
    k_int[:, [381, 382, 383, 509, 510, 511]] = 0.0

    # kT arr: [core][13, 128, 2(h), 2(dc), 2(i), 512(n)]
    #   <- k_int[c*12800 + (2w+h)*512 + n, dc*256 + i*128 + p]; chunk 26 = pad
    k_ext = np.concatenate(
        [k_int.reshape(NC_CORES, 25, 512, D),
         np.full((NC_CORES, 1, 512, D), -30.0, np.float32)], axis=1)
    karr = k_ext.reshape(NC_CORES, 13, 2, 512, 2, 2, 128).transpose(0, 1, 6, 2, 4, 5, 3)
    karr = np.ascontiguousarray(karr)                   # [c, w, p, h, dc, i, n]
    nw = (np.arange(2)[:, None] * 512 + np.arange(512)[None, :]).astype(np.float32)
    karr[:, :, 125, :, 1, 0, :] = 192.0                 # q 128     -> +24576
    karr[:, :, 125, :, 1, 1, :] = 0.0
    karr[:, :, 126, :, 1, 0, :] = np.floor(nw / 256)    # q 2^-1    -> a*2^-1
    karr[:, :, 126, :, 1, 1, :] = np.floor(nw / 16) % 16  # q 2^-5  -> b*2^-5
    karr[:, :, 127, :, 1, 0, :] = nw % 16               # q 2^-9    -> c*2^-9
    karr[:, :, 127, :, 1, 1, :] = 0.0
    karr = karr.astype(ml_dtypes.float8_e4m3)
    # qT arr: [2, 128, 2, 256] <- q_int[q, dc*256+i*128+p]
    qarr = q_int.T.reshape(2, 2, 128, 2, 128).transpose(3, 2, 0, 1, 4)
    qarr = np.ascontiguousarray(qarr)                   # [qt, p, dc, i, q]
    qarr[:, 125, 1, 0, :] = 128.0
    qarr[:, 125, 1, 1, :] = 0.0
    qarr[:, 126, 1, 0, :] = 0.5
    qarr[:, 126, 1, 1, :] = 2.0 ** -5
    qarr[:, 127, 1, 0, :] = 2.0 ** -9
    qarr[:, 127, 1, 1, :] = 0.0
    qarr = qarr.astype(ml_dtypes.float8_e4m3)

    ph1 = _phase1_nc()
    in_maps = [{"kT": karr[c], "qT": qarr} for c in range(NC_CORES)]
    res1 = run_bass_kernel_spmd(ph1, in_maps, core_ids=list(range(NC_CORES)))

    # ---- host: decode candidates, exact re-score, global top-32 ----
    cand_r = []   # row indices
    cand_k = []   # global key indices
    win_base = np.zeros(56, np.int64)       # l1 col -> window base (in-shard)
    win_base[0:8] = 6 * WIN
    for j in range(5):
        win_base[8 + 8 * j:16 + 8 * j] = (7 + j) * WIN
    win_base[48:56] = 12 * WIN
    rows128 = np.arange(128)
    for c in range(NC_CORES):
        l1 = res1.results[c]["l1"].view(np.uint32)      # [2, 128, 56]
        tk = res1.results[c]["tk"]                      # [2, NREG, 128, 32]
        for qt in range(2):
            # DVE path: packed low-10-bit in-window index
            ks = c * SHARD + win_base[None, :] + (l1[qt] & np.uint32(0x3FF))
            cand_k.append(ks.reshape(-1))
            cand_r.append(np.repeat(qt * 128 + rows128, 56))
            # topk path: flat idx within [16, TOPW] token slab
            for r in range(NREG):
                idx = tk[qt, r, :, 16:32].astype(np.int64).reshape(8, 256)
                p_rel = idx // TOPW
                col = idx % TOPW
                tok = np.arange(8)[:, None]
                rows = qt * 128 + tok * 16 + p_rel
                keys = c * SHARD + r * TOPW + col
                cand_r.append(rows.reshape(-1))
                cand_k.append(keys.reshape(-1))
    cand_r = np.concatenate(cand_r)
    cand_k = np.concatenate(cand_k)
    keep = cand_k < N
    cand_r = cand_r[keep]
    cand_k = cand_k[keep].astype(np.int64)

    # per-row candidate matrix (padded with key 0 dups; ordered by key index
    # for reference-stable tie-breaking)
    order = np.lexsort((cand_k, cand_r))
    cand_r = cand_r[order]
    cand_k = cand_k[order]
    counts = np.bincount(cand_r, minlength=B)
    maxc = int(counts.max())
    grid = np.zeros((B, maxc), np.int64)
    mask = np.zeros((B, maxc), bool)
    pos = (np.arange(cand_r.size) -
           np.concatenate([[0], np.cumsum(counts)[:-1]])[cand_r])
    grid[cand_r, pos] = cand_k
    mask[cand_r, pos] = True

    q32 = np.maximum(query_feat, 0)
    cand_keys = memory_keys[grid]                       # [B, maxc, D]
    dots = np.einsum("bd,bcd->bc", q32, cand_keys, optimize=True)
    cos = dots / np.maximum(
        np.linalg.norm(q32, axis=1)[:, None] * kn[grid], np.float32(1e-8))
    cos[~mask] = -np.inf
    # dedup: same key may arrive from both paths; keep first occurrence
    dup = np.zeros_like(mask)
    dup[:, 1:] = grid[:, 1:] == grid[:, :-1]
    cos[dup & mask] = -np.inf
    sel = np.argsort(-cos, axis=1, kind="stable")[:, :K]
    return np.take_along_axis(grid, sel, axis=1)        # [256, 32]


def kernel(query_feat, memory_keys, Wq, bq, Wm, bm, Ws, bs, Wc, bc):
    query_feat = np.asarray(query_feat, np.float32)
    memory_keys = np.asarray(memory_keys, np.float32)
    top_idx = _knn_top32(query_feat, memory_keys)
    knn = memory_keys[top_idx]                          # [256, 32, 512]

    # ---- phase 2 (batch sharded) ----
    ph2 = _phase2_nc()
    b16 = ml_dtypes.bfloat16
    bqm = (np.asarray(bq, np.float32) + np.asarray(bm, np.float32)).reshape(A, 1)
    Wq_a = np.asarray(Wq, np.float32).astype(b16)
    Wm_a = np.asarray(Wm, np.float32).astype(b16)
    Ws_a = np.asarray(Ws, np.float32).astype(b16)
    Wc_a = np.asarray(Wc, np.float32).astype(b16)
    S = (np.arange(BROWS)[:, None] == (np.arange(BROWS * K)[None, :] // K)).astype(b16)
    p128 = np.arange(128)
    tj = np.arange(256)
    m256 = ((tj[None, :] % 32) == (4 * (tj[None, :] // 32) + p128[:, None] // 32)
            ).astype(b16)
    qr = np.maximum(query_feat, 0).astype(np.float32)
    in_maps2 = []
    for c in range(NC_CORES):
        rows = slice(c * BROWS, (c + 1) * BROWS)
        knn_c = knn[rows].reshape(BROWS * K, D)
        in_maps2.append({
            "qTr": np.ascontiguousarray(qr[rows].T).astype(b16),
            "knn": knn_c.astype(b16),
            "knnT": np.ascontiguousarray(knn_c.T).astype(b16),
            "Wq": Wq_a, "Wm": Wm_a, "Ws": Ws_a, "bqm": bqm, "Wc": Wc_a,
            "S": S, "m256": m256,
        })
    res2 = run_bass_kernel_spmd(ph2, in_maps2, core_ids=list(range(NC_CORES)))
    out = np.concatenate([res2.results[c]["out"] for c in range(NC_CORES)], axis=0)
    return (out + np.asarray(bc, np.float32)[None, :]).astype(np.float32)


# revision 22
# speedup vs baseline: 2.5431x; 1.0851x over previous
"""Trainium2 Bass kernel for retrieval-knn attention classifier (nn_MA_51866025067137).

Strategy (8 NeuronCores):
  Phase 1 — memory_keys sharded along N (12800 keys/core, padded 100000->102400).
  Keys/queries are quantized to small integers and fed to fp8e4 DoubleRow
  matmuls (2 rows/cycle on the PE).  A 3-row fp32r "bias" matmul adds
  BIG + iota*2^-9 so every similarity lands in one fp32 binade [2^14, 2^15)
  with its low 10 mantissa bits equal to the column index (sims are exact
  even integers, so the pack costs nothing).  Per-1024 window top-8 is then a
  single DVE max8 straight out of PSUM; the first 6400 columns instead go
  through ACT eviction + two GPSIMD topk calls (top-256 per 16-partition
  token with indices).  Candidate extraction/merging/re-scoring is done on
  the host, which gathers the exact global top-32 key vectors.
  Phase 2 — batch sharded (32 queries/core): memory-attention module
  (tanh(qWq + knnWm + b) -> scores -> softmax -> weighted sum) and classifier,
  via small fp32r matmuls; the softmax-weighted sum is a block-diagonal matmul.
"""

import numpy as np
import ml_dtypes

import concourse.bacc as bacc
import concourse.mybir as mybir
from concourse.tile import TileContext, add_dep_helper
from concourse.bass_utils import run_bass_kernel_spmd
from concourse.masks import make_identity

# problem dims (hardcoded per harness contract)
B, N, D = 256, 100000, 512
A, C, K = 256, 100, 32
NC_CORES = 8
NPAD = 102400             # 8 * 12800
SHARD = NPAD // NC_CORES  # 12800
WIN = 1024                # DVE max8 window
NWIN = 13                 # windows 0..11 full, 12 is half (512)
TOPW = 3200               # gpsimd topk region width (vocab = 16*3200 = 51200)
NREG = 2                  # topk regions per qt: cols [0, 6400)
TOPC = NREG * TOPW        # 6400 cols to ACT-evict per qt
BROWS = B // NC_CORES     # 32 rows per core in phase 2
BIG = 24576.0             # binade [2^14, 2^15); ulp 2^-9
ULP = 2.0 ** -9

f32 = mybir.dt.float32
f32r = mybir.dt.float32r
f8 = mybir.dt.float8e4
u32 = mybir.dt.uint32
bf16 = mybir.dt.bfloat16

_PH1 = None
_PH2 = None

# ---- phase-1 window plan: interleave ACT-evict (topk) and DVE-max8 windows
# so the two consumer engines run concurrently.  Evict pieces fill the two
# topk region buffers sequentially; max8 windows append 8 cols each to L1.
_EV_SRC = [(0, 0, 1024), (1, 0, 256), (2, 0, 1024), (4, 0, 1024),
           (6, 0, 1024), (8, 0, 1024), (10, 0, 1024)]   # (window, lo, len)
_MAX8_LIST = [(1, 256, 1024), (3, 0, 1024), (5, 0, 1024), (7, 0, 1024),
              (9, 0, 1024), (11, 0, 1024), (12, 0, 512)]  # (window, lo, hi)


def _ev_pieces():
    """(window, src_lo, src_hi, region, dst_off) with region splits applied."""
    out = []
    dst = 0
    for (w, lo, ln) in _EV_SRC:
        left = ln
        src = lo
        while left:
            r = dst // TOPW
            take = min(left, (r + 1) * TOPW - dst)
            out.append((w, src, src + take, r, dst - r * TOPW))
            dst += take
            src += take
            left -= take
    assert dst == TOPC
    return out


_EV_PIECES = _ev_pieces()



def _u(i):
    return i.ins if hasattr(i, "ins") else i


def _build_phase1():
    nc = bacc.Bacc("TRN2", target_bir_lowering=False)
    kT_d = nc.dram_tensor("kT", [13, 128, 2, 2, 2, 512], f8, kind="ExternalInput")
    qT_d = nc.dram_tensor("qT", [2, 128, 2, 2, 128], f8, kind="ExternalInput")
    l1_d = nc.dram_tensor("l1", [2, 128, 56], f32, kind="ExternalOutput")
    tk_d = nc.dram_tensor("tk", [2, NREG, 128, 32], u32, kind="ExternalOutput")

    with TileContext(nc) as tc:
        with (
            tc.tile_pool(name="const", bufs=1) as constp,
            tc.tile_pool(name="keys", bufs=6) as keyp,
            tc.tile_pool(name="l1", bufs=1) as l1p,
            tc.tile_pool(name="psum", bufs=2, space="PSUM") as psump,
        ):
            qT = [constp.tile([128, 2, 2, 128], f8, tag=f"qT{qt}", name=f"qT_t{qt}")
                  for qt in range(2)]
            for qt in range(2):
                nc.sync.dma_start(out=qT[qt][:], in_=qT_d[qt, :, :, :, :])

            # raw SBUF for gpsimd topk (per qt) + its output
            sims_sb = [[nc.alloc_sbuf_tensor(f"sims_sb{qt}_{r}", [128, TOPW], f32)
                        for r in range(NREG)] for qt in range(2)]
            tk_sb = [[nc.alloc_sbuf_tensor(f"tk_sb{qt}_{r}", [128, 32], u32)
                      for r in range(NREG)] for qt in range(2)]

            L1 = [l1p.tile([128, 56], f32, tag=f"l1_{qt}", name=f"l1_{qt}")
                  for qt in range(2)]
            evicts = [[[], []], [[], []]]  # [qt][region]: ACT evicts feeding topk

            ev_by_w = {}
            for (w, lo, hi, r, doff) in _EV_PIECES:
                ev_by_w.setdefault(w, []).append((lo, hi, r, doff))
            max8_by_w = {w: (lo, hi) for (w, lo, hi) in _MAX8_LIST}
            l1_off = {w: 8 * i for i, (w, lo, hi) in enumerate(_MAX8_LIST)}

            for w in range(NWIN):
                wcols = 512 if w == 12 else WIN
                nchunk = wcols // 512
                kt = keyp.tile([128, 2, 2, 2, 512], f8, tag="kt", name="kt_t")
                nc.sync.dma_start(out=kt[:], in_=kT_d[w, :, :, :, :, :])
                for qt in range(2):
                    ps = psump.tile([128, WIN], f32, tag=f"win{qt}", name=f"ps{qt}")
                    for h in range(nchunk):
                        sl = slice(h * 512, (h + 1) * 512)
                        for dc in range(2):
                            nc.tensor.matmul(
                                ps[:, sl],
                                lhsT=qT[qt][:, dc, :, :],
                                rhs=kt[:, h, dc, :, :],
                                start=(dc == 0), stop=(dc == 1),
                                perf_mode=mybir.MatmulPerfMode.DoubleRow)
                    for (lo, hi, r, doff) in ev_by_w.get(w, []):
                        ev = nc.scalar.copy(
                            out=sims_sb[qt][r][:, doff:doff + hi - lo],
                            in_=ps[:, lo:hi])
                        evicts[qt][r].append(ev)
                    if w in max8_by_w:
                        lo, hi = max8_by_w[w]
                        o = l1_off[w]
                        nc.vector.max(out=L1[qt][:, o:o + 8], in_=ps[:, lo:hi])

            for qt in range(2):
                for r in range(NREG):
                    tki = nc.gpsimd.topk(
                        tk_sb[qt][r][:], sims_sb[qt][r][:],
                        tokens=8, vocab_size=16 * TOPW, k=256)
                    for ev in evicts[qt][r]:
                        add_dep_helper(_u(tki), _u(ev), reason="topk waits evicts")
                    do = nc.sync.dma_start(out=tk_d[qt, r, :, :],
                                           in_=tk_sb[qt][r][:])
                    add_dep_helper(_u(do), _u(tki), reason="tk out waits topk")
                nc.sync.dma_start(out=l1_d[qt, :, :], in_=L1[qt][:])
    nc.finalize()
    return nc


def _build_phase2():
    nc = bacc.Bacc("TRN2", target_bir_lowering=False)
    NCD = BROWS * K  # 1024
    qTr_in = nc.dram_tensor("qTr", [D, BROWS], bf16, kind="ExternalInput")    # relu'd
    knnT_in = nc.dram_tensor("knnT", [D, NCD], bf16, kind="ExternalInput")
    knn_in = nc.dram_tensor("knn", [NCD, D], bf16, kind="ExternalInput")
    Wq_in = nc.dram_tensor("Wq", [D, A], bf16, kind="ExternalInput")
    Wm_in = nc.dram_tensor("Wm", [D, A], bf16, kind="ExternalInput")
    Ws_in = nc.dram_tensor("Ws", [A, 1], bf16, kind="ExternalInput")
    bqm_in = nc.dram_tensor("bqm", [A, 1], f32, kind="ExternalInput")         # bq+bm
    Wc_in = nc.dram_tensor("Wc", [2 * D, C], bf16, kind="ExternalInput")
    S_in = nc.dram_tensor("S", [BROWS, NCD], bf16, kind="ExternalInput")      # S[b,(b',k)]=d_bb'
    m256_in = nc.dram_tensor("m256", [128, 256], bf16, kind="ExternalInput")
    out_d = nc.dram_tensor("out", [BROWS, C], f32, kind="ExternalOutput")     # +bc host

    with TileContext(nc) as tc:
        with (
            tc.tile_pool(name="big", bufs=1) as bigp,
            tc.tile_pool(name="small", bufs=1) as smallp,
            tc.tile_pool(name="psum", bufs=1, space="PSUM") as psump,
        ):
            # ---- loads, ordered for earliest compute start ----
            qTr = smallp.tile([128, 4, BROWS], bf16, tag="qTr")
            nc.sync.dma_start(out=qTr[:],
                              in_=qTr_in[:].rearrange("(dc p) b -> p dc b", p=128))
            Wqall = smallp.tile([128, 4, A], bf16, tag="Wqall")
            nc.sync.dma_start(out=Wqall[:],
                              in_=Wq_in[:].rearrange("(dc p) a -> p dc a", p=128))
            Wmall = smallp.tile([128, 4, A], bf16, tag="Wmall")
            nc.sync.dma_start(out=Wmall[:],
                              in_=Wm_in[:].rearrange("(dc p) a -> p dc a", p=128))
            knnTall = bigp.tile([128, 4, NCD], bf16, tag="knnTall")
            for dc in range(4):
                nc.sync.dma_start(out=knnTall[:, dc, :],
                                  in_=knnT_in[dc * 128:(dc + 1) * 128, :])
            S = smallp.tile([BROWS, NCD], bf16, tag="S")
            nc.sync.dma_start(out=S[:], in_=S_in[:, :])
            Ws = [smallp.tile([128, 1], bf16, tag=f"Ws{at}", name=f"Wst{at}")
                  for at in range(2)]
            bqm = [smallp.tile([128, 1], f32, tag=f"bqm{at}", name=f"bqmt{at}")
                   for at in range(2)]
            for at in range(2):
                nc.sync.dma_start(out=Ws[at][:], in_=Ws_in[at * 128:(at + 1) * 128, :])
                nc.sync.dma_start(out=bqm[at][:], in_=bqm_in[at * 128:(at + 1) * 128, :])
            m256 = smallp.tile([128, 256], bf16, tag="m256")
            nc.sync.dma_start(out=m256[:], in_=m256_in[:, :])
            knnall = bigp.tile([128, 8, D], bf16, tag="knnall")
            nc.sync.dma_start(out=knnall[:],
                              in_=knn_in[:].rearrange("(t p) d -> p t d", p=128))
            Wcall = smallp.tile([128, 8, C], bf16, tag="Wcall")
            nc.sync.dma_start(out=Wcall[:],
                              in_=Wc_in[:].rearrange("(m p) j -> p m j", p=128))
            ones = smallp.tile([128, 1], bf16, tag="ones")
            nc.vector.memset(ones[:].bitcast(mybir.dt.uint16), 0x3F80)
            ident = smallp.tile([128, 128], f32, tag="ident")
            make_identity(nc, ident[:])

            # ---- qproj [32b, 256a] = qTr.T @ Wq ----
            qp_ps = psump.tile([BROWS, A], f32, tag="ps_misc", bufs=2, name="qp_ps")
            for dc in range(4):
                nc.tensor.matmul(qp_ps[:], lhsT=qTr[:, dc, :], rhs=Wqall[:, dc, :],
                                 start=(dc == 0), stop=(dc == 3))
            qproj = smallp.tile([BROWS, A], bf16, tag="qproj")
            nc.vector.tensor_scalar_mul(qproj[:], qp_ps[:], 1.0)

            # ---- h^T = tanh(Wm.T knn.T + qproj via S + bqm); scores ----
            sc_ps = psump.tile([1, NCD], f32, tag="ps_sc")
            hT = [bigp.tile([128, NCD], bf16, tag=f"hT{at}", name=f"hTt{at}")
                  for at in range(2)]
            for at in range(2):
                kp = psump.tile([128, NCD], f32, tag="ps_kp", bufs=2)
                for dc in range(4):
                    for half in range(2):
                        nc.tensor.matmul(
                            kp[:, half * 512:(half + 1) * 512],
                            lhsT=Wmall[:, dc, at * 128:(at + 1) * 128],
                            rhs=knnTall[:, dc, half * 512:(half + 1) * 512],
                            start=(dc == 0), stop=False)
                for half in range(2):
                    nc.tensor.matmul(
                        kp[:, half * 512:(half + 1) * 512],
                        lhsT=qproj[:, at * 128:(at + 1) * 128],
                        rhs=S[:, half * 512:(half + 1) * 512],
                        start=False, stop=(half == 1))
                nc.scalar.activation(hT[at][:], kp[:],
                                     mybir.ActivationFunctionType.Tanh,
                                     bias=bqm[at][:])
                for half in range(2):
                    nc.tensor.matmul(
                        sc_ps[:, half * 512:(half + 1) * 512],
                        lhsT=Ws[at][:],
                        rhs=hT[at][:, half * 512:(half + 1) * 512],
                        start=(at == 0), stop=(at == 1))
            e_row = smallp.tile([1, NCD], f32, tag="e_row")
            nc.scalar.activation(e_row[:], sc_ps[:1, :],
                                 mybir.ActivationFunctionType.Exp)
            # [1, 1024] -> [128, 8] via 8 PE transposes
            ec_ps = psump.tile([128, 8], f32, tag="ps_misc", bufs=2, name="ec_ps")
            for t in range(8):
                nc.tensor.transpose(ec_ps[:, t:t + 1], e_row[:1, t * 128:(t + 1) * 128],
                                    ident[:1, :1])

            # ---- w2[p, (t,j)] = e_col[p, t] * m256 ; den; attT ----
            w2 = bigp.tile([128, 256], bf16, tag="w2")
            eb = ec_ps[:, :, None].to_broadcast([128, 8, 32])
            nc.vector.tensor_tensor(w2[:].rearrange("p (t j) -> p t j", t=8),
                                    m256[:].rearrange("p (t j) -> p t j", t=8),
                                    eb, mybir.AluOpType.mult)
            den_ps = psump.tile([BROWS, 1], f32, tag="ps_misc", bufs=2, name="den_ps")
            for t in range(8):
                nc.tensor.matmul(den_ps[:], lhsT=w2[:, t * 32:(t + 1) * 32],
                                 rhs=ones[:], start=(t == 0), stop=(t == 7))
            rden = smallp.tile([BROWS, 1], f32, tag="rden")
            nc.vector.reciprocal(rden[:], den_ps[:])
            attT = smallp.tile([128, 4, BROWS], bf16, tag="attT")
            for dc in range(4):
                att_ps = psump.tile([128, BROWS], f32, tag="ps_misc", bufs=2,
                                    name=f"att_ps{dc}")
                for t in range(8):
                    nc.tensor.matmul(att_ps[:],
                                     lhsT=knnall[:, t, dc * 128:(dc + 1) * 128],
                                     rhs=w2[:, t * 32:(t + 1) * 32],
                                     start=(t == 0), stop=(t == 7))
                nc.vector.tensor_scalar_mul(attT[:, dc, :], att_ps[:], 1.0)

            # ---- classifier: out = q-part + rden * att-part (att unnormalized) ----
            out1 = psump.tile([BROWS, C], f32, tag="ps_misc", bufs=2, name="out1")
            out2 = psump.tile([BROWS, C], f32, tag="ps_misc", bufs=2, name="out2")
            for dc in range(4):
                nc.tensor.matmul(out1[:], lhsT=qTr[:, dc, :], rhs=Wcall[:, dc, :],
                                 start=(dc == 0), stop=(dc == 3))
                nc.tensor.matmul(out2[:], lhsT=attT[:, dc, :], rhs=Wcall[:, 4 + dc, :],
                                 start=(dc == 0), stop=(dc == 3))
            out1_sb = smallp.tile([BROWS, C], f32, tag="out1_sb")
            nc.vector.tensor_scalar_mul(out1_sb[:], out1[:], 1.0)
            out_sb = smallp.tile([BROWS, C], f32, tag="out_sb")
            nc.vector.scalar_tensor_tensor(
                out=out_sb[:], in0=out2[:], scalar=rden[:], in1=out1_sb[:],
                op0=mybir.AluOpType.mult, op1=mybir.AluOpType.add)
            nc.sync.dma_start(out=out_d[:, :], in_=out_sb[:])
    nc.finalize()
    return nc


def _phase1_nc():
    global _PH1
    if _PH1 is None:
        _PH1 = _build_phase1()
    return _PH1


def _phase2_nc():
    global _PH2
    if _PH2 is None:
        _PH2 = _build_phase2()
    return _PH2


def _quantize(query_feat, khat_pad):
    """Integer-quantize relu(q) and khat so that fp8e4 DoubleRow matmuls are
    exact and |sim| stays < 8192 (one fp32 binade under BIG)."""
    q32 = np.maximum(query_feat, 0)
    sq = 16.0 / q32.max()
    sk = 15.0 / np.abs(khat_pad).max()
    while True:
        q_int = np.rint(q32 * sq).astype(np.float32)            # 0..16
        k_int = 2.0 * np.rint(khat_pad * sk).astype(np.float32)  # even, |.|<=30
        qn = np.linalg.norm(q_int, axis=1).max()
        kn = np.linalg.norm(k_int, axis=1).max()
        if qn * kn < 8100.0:
            return q_int, k_int
        sq *= 0.95
        sk *= 0.97


def _knn_top32(query_feat, memory_keys):
    """Phase 1 on device + host merge: exact global top-32 indices [B, K]."""
    # ---- host prep: pad + normalize + quantize + rearrange keys ----
    kn = np.sqrt((memory_keys ** 2).sum(axis=1))
    khat = memory_keys * (1.0 / kn)[:, None]
    pad = np.full((NPAD - N, D), -1.0 / np.sqrt(D), np.float32)
    khat_pad = np.concatenate([khat.astype(np.float32), pad], axis=0)
    q_int, k_int = _quantize(query_feat, khat_pad)
    # dims {381..383, 509..511} are repurposed as bias rows: value =
    # BIG + nw*2^-9 where nw = in-window column (0..1023).  The PE sums each
    # DoubleRow (i=0,i=1) pair in ~fp16 before fp32 PSUM, so each pair-sum
    # must be fp16-exact: BIG alone, (a,b) together, c alone.
    q_int[:, [381, 382, 383, 509, 510, 511]] = 0.0
    k_int[:, [381, 382, 383, 509, 510, 511]] = 0.0

    # kT arr: [core][13, 128, 2(h), 2(dc), 2(i), 512(n)]
    #   <- k_int[c*12800 + (2w+h)*512 + n, dc*256 + i*128 + p]; chunk 26 = pad
    k_ext = np.concatenate(
        [k_int.reshape(NC_CORES, 25, 512, D),
         np.full((NC_CORES, 1, 512, D), -30.0, np.float32)], axis=1)
    karr = k_ext.reshape(NC_CORES, 13, 2, 512, 2, 2, 128).transpose(0, 1, 6, 2, 4, 5, 3)
    karr = np.ascontiguousarray(karr)                   # [c, w, p, h, dc, i, n]
    nw = (np.arange(2)[:, None] * 512 + np.arange(512)[None, :]).astype(np.float32)
    karr[:, :, 125, :, 1, 0, :] = 192.0                 # q 128     -> +24576
    karr[:, :, 125, :, 1, 1, :] = 0.0
    karr[:, :, 126, :, 1, 0, :] = np.floor(nw / 256)    # q 2^-1    -> a*2^-1
    karr[:, :, 126, :, 1, 1, :] = np.floor(nw / 16) % 16  # q 2^-5  -> b*2^-5
    karr[:, :, 127, :, 1, 0, :] = nw % 16               # q 2^-9    -> c*2^-9
    karr[:, :, 127, :, 1, 1, :] = 0.0
    karr = karr.astype(ml_dtypes.float8_e4m3)
    # qT arr: [2, 128, 2, 256] <- q_int[q, dc*256+i*128+p]
    qarr = q_int.T.reshape(2, 2, 128, 2, 128).transpose(3, 2, 0, 1, 4)
    qarr = np.ascontiguousarray(qarr)                   # [qt, p, dc, i, q]
    qarr[:, 125, 1, 0, :] = 128.0
    qarr[:, 125, 1, 1, :] = 0.0
    qarr[:, 126, 1, 0, :] = 0.5
    qarr[:, 126, 1, 1, :] = 2.0 ** -5
    qarr[:, 127, 1, 0, :] = 2.0 ** -9
    qarr[:, 127, 1, 1, :] = 0.0
    qarr = qarr.astype(ml_dtypes.float8_e4m3)

    ph1 = _phase1_nc()
    in_maps = [{"kT": karr[c], "qT": qarr} for c in range(NC_CORES)]
    res1 = run_bass_kernel_spmd(ph1, in_maps, core_ids=list(range(NC_CORES)))

    # ---- host: decode candidates, exact re-score, global top-32 ----
    cand_r = []   # row indices
    cand_k = []   # global key indices
    win_base = np.zeros(56, np.int64)       # l1 col -> window base (in-shard)
    for i, (w, lo, hi) in enumerate(_MAX8_LIST):
        win_base[8 * i:8 * i + 8] = w * WIN
    buf2shard = np.zeros(TOPC, np.int64)    # topk buffer col -> shard col
    for (w, lo, hi, r, doff) in _EV_PIECES:
        buf2shard[r * TOPW + doff:r * TOPW + doff + hi - lo] = \
            w * WIN + np.arange(lo, hi)
    rows128 = np.arange(128)
    for c in range(NC_CORES):
        l1 = res1.results[c]["l1"].view(np.uint32)      # [2, 128, 56]
        tk = res1.results[c]["tk"]                      # [2, NREG, 128, 32]
        for qt in range(2):
            # DVE path: packed low-10-bit in-window index
            ks = c * SHARD + win_base[None, :] + (l1[qt] & np.uint32(0x3FF))
            cand_k.append(ks.reshape(-1))
            cand_r.append(np.repeat(qt * 128 + rows128, 56))
            # topk path: flat idx within [16, TOPW] token slab
            for r in range(NREG):
                idx = tk[qt, r, :, 16:32].astype(np.int64).reshape(8, 256)
                p_rel = idx // TOPW
                col = idx % TOPW
                tok = np.arange(8)[:, None]
                rows = qt * 128 + tok * 16 + p_rel
                keys = c * SHARD + buf2shard[r * TOPW + col]
                cand_r.append(rows.reshape(-1))
                cand_k.append(keys.reshape(-1))
    cand_r = np.concatenate(cand_r)
    cand_k = np.concatenate(cand_k)
    keep = cand_k < N
    cand_r = cand_r[keep]
    cand_k = cand_k[keep].astype(np.int64)

    # per-row candidate matrix (padded with key 0 dups; ordered by key index
    # for reference-stable tie-breaking)
    order = np.lexsort((cand_k, cand_r))
    cand_r = cand_r[order]
    cand_k = cand_k[order]
    counts = np.bincount(cand_r, minlength=B)
    maxc = int(counts.max())
    grid = np.zeros((B, maxc), np.int64)
    mask = np.zeros((B, maxc), bool)
    pos = (np.arange(cand_r.size) -
           np.concatenate([[0], np.cumsum(counts)[:-1]])[cand_r])
    grid[cand_r, pos] = cand_k
    mask[cand_r, pos] = True

    q32 = np.maximum(query_feat, 0)
    cand_keys = memory_keys[grid]                       # [B, maxc, D]
    dots = np.einsum("bd,bcd->bc", q32, cand_keys, optimize=True)
    cos = dots / np.maximum(
        np.linalg.norm(q32, axis=1)[:, None] * kn[grid], np.float32(1e-8))
    cos[~mask] = -np.inf
    # dedup: same key may arrive from both paths; keep first occurrence
    dup = np.zeros_like(mask)
    dup[:, 1:] = grid[:, 1:] == grid[:, :-1]
    cos[dup & mask] = -np.inf
    sel = np.argsort(-cos, axis=1, kind="stable")[:, :K]
    return np.take_along_axis(grid, sel, axis=1)        # [256, 32]


def kernel(query_feat, memory_keys, Wq, bq, Wm, bm, Ws, bs, Wc, bc):
    query_feat = np.asarray(query_feat, np.float32)
    memory_keys = np.asarray(memory_keys, np.float32)
    top_idx = _knn_top32(query_feat, memory_keys)
    knn = memory_keys[top_idx]                          # [256, 32, 512]

    # ---- phase 2 (batch sharded) ----
    ph2 = _phase2_nc()
    b16 = ml_dtypes.bfloat16
    bqm = (np.asarray(bq, np.float32) + np.asarray(bm, np.float32)).reshape(A, 1)
    Wq_a = np.asarray(Wq, np.float32).astype(b16)
    Wm_a = np.asarray(Wm, np.float32).astype(b16)
    Ws_a = np.asarray(Ws, np.float32).astype(b16)
    Wc_a = np.asarray(Wc, np.float32).astype(b16)
    S = (np.arange(BROWS)[:, None] == (np.arange(BROWS * K)[None, :] // K)).astype(b16)
    p128 = np.arange(128)
    tj = np.arange(256)
    m256 = ((tj[None, :] % 32) == (4 * (tj[None, :] // 32) + p128[:, None] // 32)
            ).astype(b16)
    qr = np.maximum(query_feat, 0).astype(np.float32)
    in_maps2 = []
    for c in range(NC_CORES):
        rows = slice(c * BROWS, (c + 1) * BROWS)
        knn_c = knn[rows].reshape(BROWS * K, D)
        in_maps2.append({
            "qTr": np.ascontiguousarray(qr[rows].T).astype(b16),
            "knn": knn_c.astype(b16),
            "knnT": np.ascontiguousarray(knn_c.T).astype(b16),
            "Wq": Wq_a, "Wm": Wm_a, "Ws": Ws_a, "bqm": bqm, "Wc": Wc_a,
            "S": S, "m256": m256,
        })
    res2 = run_bass_kernel_spmd(ph2, in_maps2, core_ids=list(range(NC_CORES)))
    out = np.concatenate([res2.results[c]["out"] for c in range(NC_CORES)], axis=0)
    return (out + np.asarray(bc, np.float32)[None, :]).astype(np.float32)


# revision 23
# speedup vs baseline: 2.5522x; 1.0036x over previous
"""Trainium2 Bass kernel for retrieval-knn attention classifier (nn_MA_51866025067137).

Strategy (8 NeuronCores):
  Phase 1 — memory_keys sharded along N (12800 keys/core, padded 100000->102400).
  Keys/queries are quantized to small integers and fed to fp8e4 DoubleRow
  matmuls (2 rows/cycle on the PE).  A 3-row fp32r "bias" matmul adds
  BIG + iota*2^-9 so every similarity lands in one fp32 binade [2^14, 2^15)
  with its low 10 mantissa bits equal to the column index (sims are exact
  even integers, so the pack costs nothing).  Per-1024 window top-8 is then a
  single DVE max8 straight out of PSUM; the first 6400 columns instead go
  through ACT eviction + two GPSIMD topk calls (top-256 per 16-partition
  token with indices).  Candidate extraction/merging/re-scoring is done on
  the host, which gathers the exact global top-32 key vectors.
  Phase 2 — batch sharded (32 queries/core): memory-attention module
  (tanh(qWq + knnWm + b) -> scores -> softmax -> weighted sum) and classifier,
  via small fp32r matmuls; the softmax-weighted sum is a block-diagonal matmul.
"""

import numpy as np
import ml_dtypes

import concourse.bacc as bacc
import concourse.mybir as mybir
from concourse.tile import TileContext, add_dep_helper
from concourse.bass_utils import run_bass_kernel_spmd
from concourse.masks import make_identity

# problem dims (hardcoded per harness contract)
B, N, D = 256, 100000, 512
A, C, K = 256, 100, 32
NC_CORES = 8
NPAD = 102400             # 8 * 12800
SHARD = NPAD // NC_CORES  # 12800
WIN = 1024                # DVE max8 window
NWIN = 13                 # windows 0..11 full, 12 is half (512)
TOPW = 3200               # gpsimd topk region width (vocab = 16*3200 = 51200)
NREG = 2                  # topk regions per qt: cols [0, 6400)
TOPC = NREG * TOPW        # 6400 cols to ACT-evict per qt
BROWS = B // NC_CORES     # 32 rows per core in phase 2
BIG = 24576.0             # binade [2^14, 2^15); ulp 2^-9
ULP = 2.0 ** -9

f32 = mybir.dt.float32
f32r = mybir.dt.float32r
f8 = mybir.dt.float8e4
u32 = mybir.dt.uint32
bf16 = mybir.dt.bfloat16

_PH1 = None
_PH2 = None

# ---- phase-1 window plan: interleave ACT-evict (topk) and DVE-max8 windows
# so the two consumer engines run concurrently.  Evict pieces fill the two
# topk region buffers sequentially; max8 windows append 8 cols each to L1.
_EV_SRC = [(0, 0, 1024), (1, 0, 256), (2, 0, 1024), (4, 0, 1024),
           (6, 0, 1024), (8, 0, 1024), (10, 0, 1024)]   # (window, lo, len)
_MAX8_LIST = [(1, 256, 1024), (3, 0, 1024), (5, 0, 1024), (7, 0, 1024),
              (9, 0, 1024), (11, 0, 1024), (12, 0, 512)]  # (window, lo, hi)


def _ev_pieces():
    """(window, src_lo, src_hi, region, dst_off) with region splits applied."""
    out = []
    dst = 0
    for (w, lo, ln) in _EV_SRC:
        left = ln
        src = lo
        while left:
            r = dst // TOPW
            take = min(left, (r + 1) * TOPW - dst)
            out.append((w, src, src + take, r, dst - r * TOPW))
            dst += take
            src += take
            left -= take
    assert dst == TOPC
    return out


_EV_PIECES = _ev_pieces()



def _u(i):
    return i.ins if hasattr(i, "ins") else i


def _build_phase1():
    nc = bacc.Bacc("TRN2", target_bir_lowering=False)
    kT_d = nc.dram_tensor("kT", [13, 128, 2, 2, 2, 512], f8, kind="ExternalInput")
    qT_d = nc.dram_tensor("qT", [2, 128, 2, 2, 128], f8, kind="ExternalInput")
    l1_d = nc.dram_tensor("l1", [2, 128, 56], f32, kind="ExternalOutput")
    tk_d = nc.dram_tensor("tk", [2, NREG, 128, 32], u32, kind="ExternalOutput")

    with TileContext(nc) as tc:
        with (
            tc.tile_pool(name="const", bufs=1) as constp,
            tc.tile_pool(name="keys", bufs=6) as keyp,
            tc.tile_pool(name="l1", bufs=1) as l1p,
            tc.tile_pool(name="psum", bufs=2, space="PSUM") as psump,
        ):
            qT = [constp.tile([128, 2, 2, 128], f8, tag=f"qT{qt}", name=f"qT_t{qt}")
                  for qt in range(2)]
            for qt in range(2):
                nc.sync.dma_start(out=qT[qt][:], in_=qT_d[qt, :, :, :, :])

            # raw SBUF for gpsimd topk (per qt) + its output
            sims_sb = [[nc.alloc_sbuf_tensor(f"sims_sb{qt}_{r}", [128, TOPW], f32)
                        for r in range(NREG)] for qt in range(2)]
            tk_sb = [[nc.alloc_sbuf_tensor(f"tk_sb{qt}_{r}", [128, 32], u32)
                      for r in range(NREG)] for qt in range(2)]

            L1 = [l1p.tile([128, 56], f32, tag=f"l1_{qt}", name=f"l1_{qt}")
                  for qt in range(2)]
            evicts = [[[], []], [[], []]]  # [qt][region]: ACT evicts feeding topk

            ev_by_w = {}
            for (w, lo, hi, r, doff) in _EV_PIECES:
                ev_by_w.setdefault(w, []).append((lo, hi, r, doff))
            max8_by_w = {w: (lo, hi) for (w, lo, hi) in _MAX8_LIST}
            l1_off = {w: 8 * i for i, (w, lo, hi) in enumerate(_MAX8_LIST)}

            for w in range(NWIN):
                wcols = 512 if w == 12 else WIN
                nchunk = wcols // 512
                kt = keyp.tile([128, 2, 2, 2, 512], f8, tag="kt", name="kt_t")
                nc.sync.dma_start(out=kt[:], in_=kT_d[w, :, :, :, :, :])
                for qt in range(2):
                    ps = psump.tile([128, WIN], f32, tag=f"win{qt}", name=f"ps{qt}")
                    for h in range(nchunk):
                        sl = slice(h * 512, (h + 1) * 512)
                        for dc in range(2):
                            nc.tensor.matmul(
                                ps[:, sl],
                                lhsT=qT[qt][:, dc, :, :],
                                rhs=kt[:, h, dc, :, :],
                                start=(dc == 0), stop=(dc == 1),
                                perf_mode=mybir.MatmulPerfMode.DoubleRow)
                    for (lo, hi, r, doff) in ev_by_w.get(w, []):
                        ev = nc.scalar.copy(
                            out=sims_sb[qt][r][:, doff:doff + hi - lo],
                            in_=ps[:, lo:hi])
                        evicts[qt][r].append(ev)
                    if w in max8_by_w:
                        lo, hi = max8_by_w[w]
                        o = l1_off[w]
                        nc.vector.max(out=L1[qt][:, o:o + 8], in_=ps[:, lo:hi])

            for qt in range(2):
                for r in range(NREG):
                    tki = nc.gpsimd.topk(
                        tk_sb[qt][r][:], sims_sb[qt][r][:],
                        tokens=8, vocab_size=16 * TOPW, k=256)
                    for ev in evicts[qt][r]:
                        add_dep_helper(_u(tki), _u(ev), reason="topk waits evicts")
                    do = nc.sync.dma_start(out=tk_d[qt, r, :, :],
                                           in_=tk_sb[qt][r][:])
                    add_dep_helper(_u(do), _u(tki), reason="tk out waits topk")
                nc.sync.dma_start(out=l1_d[qt, :, :], in_=L1[qt][:])
    nc.finalize()
    return nc


def _build_phase2():
    nc = bacc.Bacc("TRN2", target_bir_lowering=False)
    NCD = BROWS * K  # 1024
    qTr_in = nc.dram_tensor("qTr", [D, BROWS], bf16, kind="ExternalInput")    # relu'd
    knnT_in = nc.dram_tensor("knnT", [D, NCD], bf16, kind="ExternalInput")
    knn_in = nc.dram_tensor("knn", [NCD, D], bf16, kind="ExternalInput")
    Wqm_in = nc.dram_tensor("Wqm", [D, 2 * A], bf16, kind="ExternalInput")
    Ws_in = nc.dram_tensor("Ws", [A, 1], bf16, kind="ExternalInput")
    bqm_in = nc.dram_tensor("bqm", [A, 1], f32, kind="ExternalInput")         # bq+bm
    Wc_in = nc.dram_tensor("Wc", [2 * D, C], bf16, kind="ExternalInput")
    S_in = nc.dram_tensor("S", [BROWS, NCD], bf16, kind="ExternalInput")      # S[b,(b',k)]=d_bb'
    m256_in = nc.dram_tensor("m256", [128, 256], bf16, kind="ExternalInput")
    out_d = nc.dram_tensor("out", [BROWS, C], f32, kind="ExternalOutput")     # +bc host

    with TileContext(nc) as tc:
        with (
            tc.tile_pool(name="big", bufs=1) as bigp,
            tc.tile_pool(name="small", bufs=1) as smallp,
            tc.tile_pool(name="psum", bufs=1, space="PSUM") as psump,
        ):
            # ---- loads, ordered for earliest compute start ----
            qTr = smallp.tile([128, 4, BROWS], bf16, tag="qTr")
            nc.sync.dma_start(out=qTr[:],
                              in_=qTr_in[:].rearrange("(dc p) b -> p dc b", p=128))
            Wqmall = smallp.tile([128, 4, 2 * A], bf16, tag="Wqmall")
            nc.sync.dma_start(out=Wqmall[:],
                              in_=Wqm_in[:].rearrange("(dc p) a -> p dc a", p=128))
            Wqall = Wqmall[:, :, :A]
            Wmall = Wqmall[:, :, A:]
            knnTall = bigp.tile([128, 4, NCD], bf16, tag="knnTall")
            for dc in range(4):
                nc.sync.dma_start(out=knnTall[:, dc, :],
                                  in_=knnT_in[dc * 128:(dc + 1) * 128, :])
            S = smallp.tile([BROWS, NCD], bf16, tag="S")
            nc.sync.dma_start(out=S[:], in_=S_in[:, :])
            Ws = [smallp.tile([128, 1], bf16, tag=f"Ws{at}", name=f"Wst{at}")
                  for at in range(2)]
            bqm = [smallp.tile([128, 1], f32, tag=f"bqm{at}", name=f"bqmt{at}")
                   for at in range(2)]
            for at in range(2):
                nc.sync.dma_start(out=Ws[at][:], in_=Ws_in[at * 128:(at + 1) * 128, :])
                nc.sync.dma_start(out=bqm[at][:], in_=bqm_in[at * 128:(at + 1) * 128, :])
            m256 = smallp.tile([128, 256], bf16, tag="m256")
            nc.sync.dma_start(out=m256[:], in_=m256_in[:, :])
            knnall = bigp.tile([128, 8, D], bf16, tag="knnall")
            nc.sync.dma_start(out=knnall[:],
                              in_=knn_in[:].rearrange("(t p) d -> p t d", p=128))
            Wcall = smallp.tile([128, 8, C], bf16, tag="Wcall")
            nc.sync.dma_start(out=Wcall[:],
                              in_=Wc_in[:].rearrange("(m p) j -> p m j", p=128))
            ones = smallp.tile([128, 1], bf16, tag="ones")
            nc.vector.memset(ones[:].bitcast(mybir.dt.uint16), 0x3F80)
            ident = smallp.tile([128, 128], f32, tag="ident")
            make_identity(nc, ident[:])

            # ---- qproj [32b, 256a] = qTr.T @ Wq ----
            qp_ps = psump.tile([BROWS, A], f32, tag="ps_misc", bufs=2, name="qp_ps")
            for dc in range(4):
                nc.tensor.matmul(qp_ps[:], lhsT=qTr[:, dc, :], rhs=Wqall[:, dc, :],
                                 start=(dc == 0), stop=(dc == 3))
            qproj = smallp.tile([BROWS, A], bf16, tag="qproj")
            nc.vector.tensor_scalar_mul(qproj[:], qp_ps[:], 1.0)

            # ---- h^T = tanh(Wm.T knn.T + qproj via S + bqm); scores ----
            sc_ps = psump.tile([1, NCD], f32, tag="ps_sc")
            hT = [bigp.tile([128, NCD], bf16, tag=f"hT{at}", name=f"hTt{at}")
                  for at in range(2)]
            for at in range(2):
                kp = psump.tile([128, NCD], f32, tag="ps_kp", bufs=2)
                for dc in range(4):
                    for half in range(2):
                        nc.tensor.matmul(
                            kp[:, half * 512:(half + 1) * 512],
                            lhsT=Wmall[:, dc, at * 128:(at + 1) * 128],
                            rhs=knnTall[:, dc, half * 512:(half + 1) * 512],
                            start=(dc == 0), stop=False)
                for half in range(2):
                    nc.tensor.matmul(
                        kp[:, half * 512:(half + 1) * 512],
                        lhsT=qproj[:, at * 128:(at + 1) * 128],
                        rhs=S[:, half * 512:(half + 1) * 512],
                        start=False, stop=(half == 1))
                nc.scalar.activation(hT[at][:], kp[:],
                                     mybir.ActivationFunctionType.Tanh,
                                     bias=bqm[at][:])
                for half in range(2):
                    nc.tensor.matmul(
                        sc_ps[:, half * 512:(half + 1) * 512],
                        lhsT=Ws[at][:],
                        rhs=hT[at][:, half * 512:(half + 1) * 512],
                        start=(at == 0), stop=(at == 1))
            e_row = smallp.tile([1, NCD], f32, tag="e_row")
            nc.scalar.activation(e_row[:], sc_ps[:1, :],
                                 mybir.ActivationFunctionType.Exp)
            # [1, 1024] -> [128, 8] via 8 PE transposes
            ec_ps = psump.tile([128, 8], f32, tag="ps_misc", bufs=2, name="ec_ps")
            for t in range(8):
                nc.tensor.transpose(ec_ps[:, t:t + 1], e_row[:1, t * 128:(t + 1) * 128],
                                    ident[:1, :1])

            # ---- w2[p, (t,j)] = e_col[p, t] * m256 ; den; attT ----
            w2 = bigp.tile([128, 256], bf16, tag="w2")
            eb = ec_ps[:, :, None].to_broadcast([128, 8, 32])
            nc.vector.tensor_tensor(w2[:].rearrange("p (t j) -> p t j", t=8),
                                    m256[:].rearrange("p (t j) -> p t j", t=8),
                                    eb, mybir.AluOpType.mult)
            den_ps = psump.tile([BROWS, 1], f32, tag="ps_misc", bufs=2, name="den_ps")
            for t in range(8):
                nc.tensor.matmul(den_ps[:], lhsT=w2[:, t * 32:(t + 1) * 32],
                                 rhs=ones[:], start=(t == 0), stop=(t == 7))
            rden = smallp.tile([BROWS, 1], f32, tag="rden")
            nc.vector.reciprocal(rden[:], den_ps[:])
            attT = smallp.tile([128, 4, BROWS], bf16, tag="attT")
            for dc in range(4):
                att_ps = psump.tile([128, BROWS], f32, tag="ps_misc", bufs=2,
                                    name=f"att_ps{dc}")
                for t in range(8):
                    nc.tensor.matmul(att_ps[:],
                                     lhsT=knnall[:, t, dc * 128:(dc + 1) * 128],
                                     rhs=w2[:, t * 32:(t + 1) * 32],
                                     start=(t == 0), stop=(t == 7))
                nc.vector.tensor_scalar_mul(attT[:, dc, :], att_ps[:], 1.0)

            # ---- classifier: out = q-part + rden * att-part (att unnormalized) ----
            out1 = psump.tile([BROWS, C], f32, tag="ps_misc", bufs=2, name="out1")
            out2 = psump.tile([BROWS, C], f32, tag="ps_misc", bufs=2, name="out2")
            for dc in range(4):
                nc.tensor.matmul(out1[:], lhsT=qTr[:, dc, :], rhs=Wcall[:, dc, :],
                                 start=(dc == 0), stop=(dc == 3))
                nc.tensor.matmul(out2[:], lhsT=attT[:, dc, :], rhs=Wcall[:, 4 + dc, :],
                                 start=(dc == 0), stop=(dc == 3))
            out1_sb = smallp.tile([BROWS, C], f32, tag="out1_sb")
            nc.vector.tensor_scalar_mul(out1_sb[:], out1[:], 1.0)
            out_sb = smallp.tile([BROWS, C], f32, tag="out_sb")
            nc.vector.scalar_tensor_tensor(
                out=out_sb[:], in0=out2[:], scalar=rden[:], in1=out1_sb[:],
                op0=mybir.AluOpType.mult, op1=mybir.AluOpType.add)
            nc.sync.dma_start(out=out_d[:, :], in_=out_sb[:])
    nc.finalize()
    return nc


def _phase1_nc():
    global _PH1
    if _PH1 is None:
        _PH1 = _build_phase1()
    return _PH1


def _phase2_nc():
    global _PH2
    if _PH2 is None:
        _PH2 = _build_phase2()
    return _PH2


def _quantize(query_feat, khat_pad):
    """Integer-quantize relu(q) and khat so that fp8e4 DoubleRow matmuls are
    exact and |sim| stays < 8192 (one fp32 binade under BIG)."""
    q32 = np.maximum(query_feat, 0)
    sq = 16.0 / q32.max()
    sk = 15.0 / np.abs(khat_pad).max()
    while True:
        q_int = np.rint(q32 * sq).astype(np.float32)            # 0..16
        k_int = 2.0 * np.rint(khat_pad * sk).astype(np.float32)  # even, |.|<=30
        qn = np.linalg.norm(q_int, axis=1).max()
        kn = np.linalg.norm(k_int, axis=1).max()
        if qn * kn < 8100.0:
            return q_int, k_int
        sq *= 0.95
        sk *= 0.97


def _knn_top32(query_feat, memory_keys):
    """Phase 1 on device + host merge: exact global top-32 indices [B, K]."""
    # ---- host prep: pad + normalize + quantize + rearrange keys ----
    kn = np.sqrt((memory_keys ** 2).sum(axis=1))
    khat = memory_keys * (1.0 / kn)[:, None]
    pad = np.full((NPAD - N, D), -1.0 / np.sqrt(D), np.float32)
    khat_pad = np.concatenate([khat.astype(np.float32), pad], axis=0)
    q_int, k_int = _quantize(query_feat, khat_pad)
    # dims {381..383, 509..511} are repurposed as bias rows: value =
    # BIG + nw*2^-9 where nw = in-window column (0..1023).  The PE sums each
    # DoubleRow (i=0,i=1) pair in ~fp16 before fp32 PSUM, so each pair-sum
    # must be fp16-exact: BIG alone, (a,b) together, c alone.
    q_int[:, [381, 382, 383, 509, 510, 511]] = 0.0
    k_int[:, [381, 382, 383, 509, 510, 511]] = 0.0

    # kT arr: [core][13, 128, 2(h), 2(dc), 2(i), 512(n)]
    #   <- k_int[c*12800 + (2w+h)*512 + n, dc*256 + i*128 + p]; chunk 26 = pad
    k_ext = np.concatenate(
        [k_int.reshape(NC_CORES, 25, 512, D),
         np.full((NC_CORES, 1, 512, D), -30.0, np.float32)], axis=1)
    karr = k_ext.reshape(NC_CORES, 13, 2, 512, 2, 2, 128).transpose(0, 1, 6, 2, 4, 5, 3)
    karr = np.ascontiguousarray(karr)                   # [c, w, p, h, dc, i, n]
    nw = (np.arange(2)[:, None] * 512 + np.arange(512)[None, :]).astype(np.float32)
    karr[:, :, 125, :, 1, 0, :] = 192.0                 # q 128     -> +24576
    karr[:, :, 125, :, 1, 1, :] = 0.0
    karr[:, :, 126, :, 1, 0, :] = np.floor(nw / 256)    # q 2^-1    -> a*2^-1
    karr[:, :, 126, :, 1, 1, :] = np.floor(nw / 16) % 16  # q 2^-5  -> b*2^-5
    karr[:, :, 127, :, 1, 0, :] = nw % 16               # q 2^-9    -> c*2^-9
    karr[:, :, 127, :, 1, 1, :] = 0.0
    karr = karr.astype(ml_dtypes.float8_e4m3)
    # qT arr: [2, 128, 2, 256] <- q_int[q, dc*256+i*128+p]
    qarr = q_int.T.reshape(2, 2, 128, 2, 128).transpose(3, 2, 0, 1, 4)
    qarr = np.ascontiguousarray(qarr)                   # [qt, p, dc, i, q]
    qarr[:, 125, 1, 0, :] = 128.0
    qarr[:, 125, 1, 1, :] = 0.0
    qarr[:, 126, 1, 0, :] = 0.5
    qarr[:, 126, 1, 1, :] = 2.0 ** -5
    qarr[:, 127, 1, 0, :] = 2.0 ** -9
    qarr[:, 127, 1, 1, :] = 0.0
    qarr = qarr.astype(ml_dtypes.float8_e4m3)

    ph1 = _phase1_nc()
    in_maps = [{"kT": karr[c], "qT": qarr} for c in range(NC_CORES)]
    res1 = run_bass_kernel_spmd(ph1, in_maps, core_ids=list(range(NC_CORES)))

    # ---- host: decode candidates, exact re-score, global top-32 ----
    cand_r = []   # row indices
    cand_k = []   # global key indices
    win_base = np.zeros(56, np.int64)       # l1 col -> window base (in-shard)
    for i, (w, lo, hi) in enumerate(_MAX8_LIST):
        win_base[8 * i:8 * i + 8] = w * WIN
    buf2shard = np.zeros(TOPC, np.int64)    # topk buffer col -> shard col
    for (w, lo, hi, r, doff) in _EV_PIECES:
        buf2shard[r * TOPW + doff:r * TOPW + doff + hi - lo] = \
            w * WIN + np.arange(lo, hi)
    rows128 = np.arange(128)
    for c in range(NC_CORES):
        l1 = res1.results[c]["l1"].view(np.uint32)      # [2, 128, 56]
        tk = res1.results[c]["tk"]                      # [2, NREG, 128, 32]
        for qt in range(2):
            # DVE path: packed low-10-bit in-window index
            ks = c * SHARD + win_base[None, :] + (l1[qt] & np.uint32(0x3FF))
            cand_k.append(ks.reshape(-1))
            cand_r.append(np.repeat(qt * 128 + rows128, 56))
            # topk path: flat idx within [16, TOPW] token slab
            for r in range(NREG):
                idx = tk[qt, r, :, 16:32].astype(np.int64).reshape(8, 256)
                p_rel = idx // TOPW
                col = idx % TOPW
                tok = np.arange(8)[:, None]
                rows = qt * 128 + tok * 16 + p_rel
                keys = c * SHARD + buf2shard[r * TOPW + col]
                cand_r.append(rows.reshape(-1))
                cand_k.append(keys.reshape(-1))
    cand_r = np.concatenate(cand_r)
    cand_k = np.concatenate(cand_k)
    keep = cand_k < N
    cand_r = cand_r[keep]
    cand_k = cand_k[keep].astype(np.int64)

    # per-row candidate matrix (padded with key 0 dups; ordered by key index
    # for reference-stable tie-breaking)
    order = np.lexsort((cand_k, cand_r))
    cand_r = cand_r[order]
    cand_k = cand_k[order]
    counts = np.bincount(cand_r, minlength=B)
    maxc = int(counts.max())
    grid = np.zeros((B, maxc), np.int64)
    mask = np.zeros((B, maxc), bool)
    pos = (np.arange(cand_r.size) -
           np.concatenate([[0], np.cumsum(counts)[:-1]])[cand_r])
    grid[cand_r, pos] = cand_k
    mask[cand_r, pos] = True

    q32 = np.maximum(query_feat, 0)
    cand_keys = memory_keys[grid]                       # [B, maxc, D]
    dots = np.einsum("bd,bcd->bc", q32, cand_keys, optimize=True)
    cos = dots / np.maximum(
        np.linalg.norm(q32, axis=1)[:, None] * kn[grid], np.float32(1e-8))
    cos[~mask] = -np.inf
    # dedup: same key may arrive from both paths; keep first occurrence
    dup = np.zeros_like(mask)
    dup[:, 1:] = grid[:, 1:] == grid[:, :-1]
    cos[dup & mask] = -np.inf
    sel = np.argsort(-cos, axis=1, kind="stable")[:, :K]
    return np.take_along_axis(grid, sel, axis=1)        # [256, 32]


def kernel(query_feat, memory_keys, Wq, bq, Wm, bm, Ws, bs, Wc, bc):
    query_feat = np.asarray(query_feat, np.float32)
    memory_keys = np.asarray(memory_keys, np.float32)
    top_idx = _knn_top32(query_feat, memory_keys)
    knn = memory_keys[top_idx]                          # [256, 32, 512]

    # ---- phase 2 (batch sharded) ----
    ph2 = _phase2_nc()
    b16 = ml_dtypes.bfloat16
    bqm = (np.asarray(bq, np.float32) + np.asarray(bm, np.float32)).reshape(A, 1)
    Wqm_a = np.concatenate([np.asarray(Wq, np.float32),
                            np.asarray(Wm, np.float32)], axis=1).astype(b16)
    Ws_a = np.asarray(Ws, np.float32).astype(b16)
    Wc_a = np.asarray(Wc, np.float32).astype(b16)
    S = (np.arange(BROWS)[:, None] == (np.arange(BROWS * K)[None, :] // K)).astype(b16)
    p128 = np.arange(128)
    tj = np.arange(256)
    m256 = ((tj[None, :] % 32) == (4 * (tj[None, :] // 32) + p128[:, None] // 32)
            ).astype(b16)
    qr = np.maximum(query_feat, 0).astype(np.float32)
    in_maps2 = []
    for c in range(NC_CORES):
        rows = slice(c * BROWS, (c + 1) * BROWS)
        knn_c = knn[rows].reshape(BROWS * K, D)
        in_maps2.append({
            "qTr": np.ascontiguousarray(qr[rows].T).astype(b16),
            "knn": knn_c.astype(b16),
            "knnT": np.ascontiguousarray(knn_c.T).astype(b16),
            "Wqm": Wqm_a, "Ws": Ws_a, "bqm": bqm, "Wc": Wc_a,
            "S": S, "m256": m256,
        })
    res2 = run_bass_kernel_spmd(ph2, in_maps2, core_ids=list(range(NC_CORES)))
    out = np.concatenate([res2.results[c]["out"] for c in range(NC_CORES)], axis=0)
    return (out + np.asarray(bc, np.float32)[None, :]).astype(np.float32)
